# revision 1
# baseline (speedup 1.0000x reference)
"""DA-RNN + batch self-attention Trainium2 kernel (8 NeuronCores, SPMD).

Strategy: data-parallel over batch (B=4096 -> 512/core) for CNN + encoder LSTM +
decoder LSTM + q/k/v projections (phase 1).  Host gathers k/v across cores, then
phase 2 computes the BxB softmax attention with score-matrix rows sharded
across cores (each core holds full softmax rows for its 512 queries).

All recurrent/attention matmuls run in bf16 with fp32 PSUM accumulation; the
small CNN runs in fp32.  Layouts are feature-major ([feature, batch]) end to
end so the LSTM recurrence and attention need no on-chip transposes.

Self-contained: hardcodes all shapes; takes the full unsharded inputs.
"""

import os
import numpy as np
import ml_dtypes
from contextlib import ExitStack

import concourse.mybir as mybir
import concourse.tile as tile
from concourse import bacc
from concourse.bass_utils import run_bass_kernel_spmd

F32 = mybir.dt.float32
BF16 = mybir.dt.bfloat16
FP8 = True                 # fp8e4m3 + DoubleRow for LSTM/attention-projection
FP8E4 = mybir.dt.float8e4
DR = mybir.MatmulPerfMode.DoubleRow
ADT = FP8E4 if FP8 else BF16   # LSTM activation storage dtype
WDT = FP8E4 if FP8 else BF16   # LSTM weight dtype
WS = 16.0 if FP8 else 1.0      # weight prescale
HS = 8.0 if FP8 else 1.0       # hidden/feat/y prescale
SC = 1.0 / (WS * HS)           # psum -> true preactivation scale
QKS = 4.0 if FP8 else 1.0      # extra prescale on stored q/k
AF = mybir.ActivationFunctionType
nbf16 = ml_dtypes.bfloat16
nfp8 = ml_dtypes.float8_e4m3
nADT = nfp8 if FP8 else nbf16

B, T, D, H, S = 4096, 45, 128, 512, 4
NCORES = 8
BL = B // NCORES          # 512 batch rows per core
BC = 128                  # CNN batch chunk
LS = [45, 23, 15, 12]     # ceil(T/s) per branch
L2 = [(l - 2) // 2 for l in LS]    # [21, 10, 6, 5]
L4 = [l - 2 for l in L2]           # [19, 8, 4, 3]
LEN = [l // 2 for l in L4]         # [9, 4, 2, 1]
TP = 9                    # downsampled sequence length
IDX = list(range(T - 1, 0, -(T // TP)))[::-1]   # [4,9,...,44]

# exec times of the two launches from the most recent kernel() call (ns or None)
LAST_EXEC_NS = [None, None]
TRACE = False
_CACHE = {}


def _build_phase1(parts=("cnn", "conv", "pool", "enc", "dec", "qkv")):
    nc = bacc.Bacc("TRN2", target_bir_lowering=False, debug=False,
                   num_devices=NCORES)
    x = nc.dram_tensor("x", [BL // BC, D, T, BC], ADT, kind="ExternalInput")
    ysel = nc.dram_tensor("ysel", [1, TP * BL], WDT, kind="ExternalInput")
    w12 = nc.dram_tensor("w12", [128, S, 3, 32], WDT, kind="ExternalInput")
    b12 = nc.dram_tensor("b12", [32, S], F32, kind="ExternalInput")
    w3d = nc.dram_tensor("w3d", [32, S, 3, 32], BF16, kind="ExternalInput")
    b3d = nc.dram_tensor("b3d", [32, S], F32, kind="ExternalInput")
    wih = nc.dram_tensor("wih", [128, 16 * 128], WDT, kind="ExternalInput")
    whh = nc.dram_tensor("whh", [128, 4, 16 * 128], WDT, kind="ExternalInput")
    bge = nc.dram_tensor("bge", [128, 16], F32, kind="ExternalInput")
    dxw = nc.dram_tensor("dxw", [128, 4, 16 * 128], WDT, kind="ExternalInput")
    dwy = nc.dram_tensor("dwy", [1, 16 * 128], WDT, kind="ExternalInput")
    dhw = nc.dram_tensor("dhw", [128, 4, 16 * 128], WDT, kind="ExternalInput")
    bgd = nc.dram_tensor("bgd", [128, 16], F32, kind="ExternalInput")
    wqt = nc.dram_tensor("wqt", [128, 4, H], WDT, kind="ExternalInput")
    wkt = nc.dram_tensor("wkt", [128, 4, H], WDT, kind="ExternalInput")
    wvl = nc.dram_tensor("wvl", [128, 4], WDT, kind="ExternalInput")
    qt_d = nc.dram_tensor("qt", [4 * 128, BL], ADT, kind="ExternalOutput")
    kt_d = nc.dram_tensor("kt", [4 * 128, BL], ADT, kind="ExternalOutput")
    vl_d = nc.dram_tensor("vl", [128, 4], BF16, kind="ExternalOutput")

    with tile.TileContext(nc) as tc, ExitStack() as ctx:
        state = ctx.enter_context(tc.tile_pool(name="state", bufs=1))
        wpool = ctx.enter_context(tc.tile_pool(name="wpool", bufs=1))
        featT = state.tile([128, TP, BL], ADT, tag="featT")
        nc.vector.memset(featT, 0.0)

        # CNN + encoder weights up front (fit alongside the CNN working set)
        w12_sb = wpool.tile([128, S, 3, 32], WDT, tag="w12")
        nc.sync.dma_start(out=w12_sb, in_=w12[:, :, :, :])
        b12_sb = wpool.tile([32, S], F32, tag="b12")
        nc.sync.dma_start(out=b12_sb, in_=b12[:, :])
        w3_sb = wpool.tile([32, S, 3, 32], BF16, tag="w3")
        nc.sync.dma_start(out=w3_sb, in_=w3d[:, :, :, :])
        b3_sb = wpool.tile([32, S], F32, tag="b3")
        nc.sync.dma_start(out=b3_sb, in_=b3d[:, :])
        wih_sb = wpool.tile([128, 16 * 128], WDT, tag="wih")
        nc.sync.dma_start(out=wih_sb, in_=wih[:, :])
        whh_sb = wpool.tile([128, 4, 16 * 128], WDT, tag="whh")
        nc.sync.dma_start(out=whh_sb, in_=whh[:, :, :])
        bge_sb = wpool.tile([128, 16], F32, tag="bge")
        nc.sync.dma_start(out=bge_sb, in_=bge[:, :])

        # ---------------- CNN downsampling (batch chunks of BC) ----------------
        with (
            tc.tile_pool(name="cnnx", bufs=1) as cnnx,
            tc.tile_pool(name="cnnh", bufs=2) as cnnh,
            tc.tile_pool(name="cnnps", bufs=4, space="PSUM") as cnnps,
        ):
            xts = []
            if "cnn" in parts:
                for ci in range(BL // BC):
                    xT = cnnx.tile([128, T, BC], ADT, tag=f"xT{ci}",
                                   name=f"xT{ci}")
                    nc.sync.dma_start(out=xT, in_=x[ci, :, :, :])
                    xts.append(xT)
            for c0 in (range(0, BL, BC) if "cnn" in parts else ()):
                xT = xts[c0 // BC]
                for s in (range(S) if "conv" in parts else ()):
                    stride = s + 1
                    l2 = L2[s]
                    nlo = 2 * l2            # conv1 outputs actually consumed
                    h3 = cnnh.tile([32, l2, BC], BF16, tag="h3")
                    for g0 in range(0, nlo, 4):
                        gn = min(4, nlo - g0)
                        ps = cnnps.tile([32, 4, BC], F32, tag="cps")
                        for sub in range(gn):
                            lo = g0 + sub
                            if FP8:
                                xpair = xT[:, lo * stride:
                                           (lo + 2) * stride, :].rearrange(
                                    "p (two r) b -> p two r b", two=2)[:, :, 0, :]
                                nc.tensor.matmul(ps[:, sub, :],
                                                 w12_sb[:, s, 0:2, :], xpair,
                                                 start=True, stop=False,
                                                 perf_mode=DR)
                                nc.tensor.matmul(ps[:, sub, :],
                                                 w12_sb[:, s, 2, :],
                                                 xT[:, (lo + 2) * stride, :],
                                                 start=False, stop=True)
                            else:
                                for k in range(3):
                                    nc.tensor.matmul(ps[:, sub, :],
                                                     w12_sb[:, s, k, :],
                                                     xT[:, (lo + k) * stride, :],
                                                     start=(k == 0),
                                                     stop=(k == 2))
                        if "pool" not in parts:
                            continue
                        # maxpool straight out of PSUM (bias folded into conv4)
                        pv = ps[:, 0:gn, :].rearrange(
                            "c (l two) b -> c l b two", two=2)
                        nc.vector.tensor_reduce(
                            h3[:, g0 // 2:(g0 + gn) // 2, :], pv,
                            mybir.AxisListType.X, mybir.AluOpType.max)
                    if "pool" not in parts:
                        continue
                    ln = LEN[s]
                    t0 = TP - ln
                    nl4 = 2 * ln            # conv4 outputs actually consumed
                    h5 = cnnh.tile([32, ln, BC], BF16, tag="h5")
                    for g0 in range(0, nl4, 4):
                        gn = min(4, nl4 - g0)
                        ps = cnnps.tile([32, 4, BC], F32, tag="cps")
                        for sub in range(gn):
                            lo = g0 + sub
                            for k in range(3):
                                nc.tensor.matmul(ps[:, sub, :],
                                                 w3_sb[:, s, k, :],
                                                 h3[:, lo + k, :],
                                                 start=(k == 0), stop=(k == 2))
                        pv = ps[:, 0:gn, :].rearrange(
                            "c (l two) b -> c l b two", two=2)
                        nc.vector.tensor_reduce(
                            h5[:, g0 // 2:(g0 + gn) // 2, :], pv,
                            mybir.AxisListType.X, mybir.AluOpType.max)
                    # featT = HS * (pooled + b3eff)  (b3_sb holds HS*b3eff)
                    nc.vector.tensor_scalar(
                        featT[32 * s:32 * (s + 1), t0:TP, c0:c0 + BC],
                        h5, HS, b3_sb[:, s:s + 1],
                        mybir.AluOpType.mult, mybir.AluOpType.add)

        gpsum = ctx.enter_context(tc.tile_pool(name="gpsum", bufs=8, space="PSUM"))
        gact = ctx.enter_context(tc.tile_pool(name="gact", bufs=10))
        gtmp = ctx.enter_context(tc.tile_pool(name="gtmp", bufs=4))
        cpool = ctx.enter_context(tc.tile_pool(name="cpool", bufs=2))
        hdpool = ctx.enter_context(tc.tile_pool(name="hdpool", bufs=2))
        # remaining weights (DMA overlaps the encoder)
        dx_sb = wpool.tile([128, 4, 16 * 128], WDT, tag="dx")
        nc.sync.dma_start(out=dx_sb, in_=dxw[:, :, :])
        dwy_sb = wpool.tile([1, 16 * 128], WDT, tag="dwy")
        nc.sync.dma_start(out=dwy_sb, in_=dwy[:, :])
        dh_sb = wpool.tile([128, 4, 16 * 128], WDT, tag="dh")
        nc.sync.dma_start(out=dh_sb, in_=dhw[:, :, :])
        bgd_sb = wpool.tile([128, 16], F32, tag="bgd")
        nc.sync.dma_start(out=bgd_sb, in_=bgd[:, :])
        wq_sb = wpool.tile([128, 4, H], WDT, tag="wq")
        nc.sync.dma_start(out=wq_sb, in_=wqt[:, :, :])
        wk_sb = wpool.tile([128, 4, H], WDT, tag="wk")
        nc.sync.dma_start(out=wk_sb, in_=wkt[:, :, :])
        wvl_sb = wpool.tile([128, 4], WDT, tag="wvl")
        nc.sync.dma_start(out=wvl_sb, in_=wvl[:, :])
        hz = state.tile([128, 4, BL], ADT, tag="hz")
        nc.vector.memset(hz, 0.0)
        hencT = state.tile([128, TP, 4, BL], ADT, tag="hencT")

        def emit_lstm(rhs_h, c_prev, h_out_full, whh_tile, bias_sb, x_mms):
            """One LSTM step, feature-major.  Gate order i,f,g,o in 4x128-row
            m-tiles.  x_mms(ps, mt) emits the input-side matmuls (first has
            start=True); the h-side k-tiles accumulate after it."""
            c_new = cpool.tile([128, 4, BL], F32, tag="c")
            for ht in range(4):
                acts = {}
                for gi, base in ((0, 0), (1, 4), (2, 8), (3, 12)):
                    if c_prev is None and gi == 1:
                        continue  # f-gate unused when initial c == 0
                    mt = base + ht
                    ps = gpsum.tile([128, BL], F32, tag="gps")
                    x_mms(ps, mt)
                    if FP8:
                        for k in (0, 2):
                            nc.tensor.matmul(
                                ps,
                                whh_tile[:, k:k + 2, mt * 128:(mt + 1) * 128],
                                rhs_h[:, k:k + 2, :], start=False,
                                stop=(k == 2), perf_mode=DR)
                    else:
                        for k in range(4):
                            nc.tensor.matmul(
                                ps, whh_tile[:, k, mt * 128:(mt + 1) * 128],
                                rhs_h[:, k, :], start=False, stop=(k == 3))
                    a = gact.tile([128, BL], BF16, tag="ga")
                    nc.scalar.activation(a, ps,
                                         AF.Tanh if gi == 2 else AF.Sigmoid,
                                         bias=bias_sb[:, mt:mt + 1], scale=SC)
                    acts[gi] = a
                if c_prev is None:
                    nc.vector.tensor_mul(c_new[:, ht, :], acts[0], acts[2])
                else:
                    t1 = gtmp.tile([128, BL], F32, tag="tt")
                    nc.vector.tensor_mul(t1, acts[1], c_prev[:, ht, :])
                    t2 = gtmp.tile([128, BL], F32, tag="tt")
                    nc.vector.tensor_mul(t2, acts[0], acts[2])
                    nc.vector.tensor_add(c_new[:, ht, :], t1, t2)
                tch = gtmp.tile([128, BL], BF16, tag="tt")
                nc.scalar.activation(tch, c_new[:, ht, :], AF.Tanh)
                if FP8:
                    nc.vector.scalar_tensor_tensor(
                        h_out_full[:, ht, :], acts[3], HS, tch,
                        mybir.AluOpType.mult, mybir.AluOpType.mult)
                else:
                    nc.vector.tensor_mul(h_out_full[:, ht, :], acts[3], tch)
            return c_new

        # ---------------- encoder ----------------
        c_prev = None
        for t in (range(TP) if "enc" in parts else ()):
            rhs_h = hz[:, :, :] if t == 0 else hencT[:, t - 1, :, :]

            def x_mms(ps, mt, _t=t):
                nc.tensor.matmul(ps, wih_sb[:, mt * 128:(mt + 1) * 128],
                                 featT[:, _t, :], start=True, stop=False)

            c_prev = emit_lstm(rhs_h, c_prev, hencT[:, t, :, :],
                               whh_sb, bge_sb, x_mms)

        # ---------------- decoder ----------------
        c_prev = None
        hd_prev = hz[:, :, :]
        ypool = ctx.enter_context(tc.tile_pool(name="ypool", bufs=2))
        for t in (range(TP) if "dec" in parts else ()):
            hd_new = hdpool.tile([128, 4, BL], ADT, tag="hd")
            yt_sb = ypool.tile([1, BL], ADT, tag="yt")
            nc.sync.dma_start(out=yt_sb, in_=ysel[0:1, t * BL:(t + 1) * BL])

            def x_mms(ps, mt, _t=t, _y=yt_sb):
                if FP8:
                    for k in (0, 2):
                        nc.tensor.matmul(
                            ps, dx_sb[:, k:k + 2, mt * 128:(mt + 1) * 128],
                            hencT[:, _t, k:k + 2, :], start=(k == 0),
                            stop=False, perf_mode=DR)
                else:
                    for k in range(4):
                        nc.tensor.matmul(ps,
                                         dx_sb[:, k, mt * 128:(mt + 1) * 128],
                                         hencT[:, _t, k, :],
                                         start=(k == 0), stop=False)
                nc.tensor.matmul(ps, dwy_sb[0:1, mt * 128:(mt + 1) * 128],
                                 _y[0:1, :], start=False, stop=False)

            c_prev = emit_lstm(hd_prev, c_prev, hd_new[:, :, :],
                               dh_sb, bgd_sb, x_mms)
            hd_prev = hd_new

        # ---------------- q/k/v projections ----------------
        if "qkv" not in parts:
            nc.compile()
            return nc
        qout = state.tile([128, 4, BL], ADT, tag="qout")
        kout = state.tile([128, 4, BL], ADT, tag="kout")
        vlout = state.tile([128, 4], BF16, tag="vlout")
        for w_sb, osb in (((wq_sb, qout), (wk_sb, kout)) if "qkv" in parts else ()):
            for mh in range(4):
                ps = gpsum.tile([128, BL], F32, tag="gps")
                if FP8:
                    for k in (0, 2):
                        nc.tensor.matmul(
                            ps, w_sb[:, k:k + 2, mh * 128:(mh + 1) * 128],
                            hd_prev[:, k:k + 2, :], start=(k == 0),
                            stop=(k == 2), perf_mode=DR)
                    nc.vector.tensor_scalar_mul(osb[:, mh, :], ps, SC * QKS)
                else:
                    for k in range(4):
                        nc.tensor.matmul(ps,
                                         w_sb[:, k, mh * 128:(mh + 1) * 128],
                                         hd_prev[:, k, :], start=(k == 0),
                                         stop=(k == 3))
                    nc.vector.tensor_copy(osb[:, mh, :], ps)
        for mi in (range(4) if "qkv" in parts else ()):
            vlps = gpsum.tile([128, BL], F32, tag="gps", name="vlps")
            for k in range(4):
                nc.tensor.matmul(vlps[:, 0:1],
                                 hd_prev[:, k, mi * 128:(mi + 1) * 128],
                                 wvl_sb[:, k:k + 1], start=(k == 0),
                                 stop=(k == 3))
            nc.vector.tensor_scalar_mul(vlout[:, mi:mi + 1], vlps[:, 0:1], SC)
        nc.sync.dma_start(out=qt_d.rearrange("(k p) i -> p k i", p=128), in_=qout)
        nc.sync.dma_start(out=kt_d.rearrange("(k p) i -> p k i", p=128), in_=kout)
        nc.sync.dma_start(out=vl_d[:, :], in_=vlout)

    nc.compile()
    return nc


def _build_phase2():
    nc = bacc.Bacc("TRN2", target_bir_lowering=False, debug=False,
                   num_devices=NCORES)
    qt = nc.dram_tensor("qt", [128, 4, BL], ADT, kind="ExternalInput")
    kb = nc.dram_tensor("kb", [B // 128, 128, 4, 128], ADT,
                        kind="ExternalInput")
    vlt = nc.dram_tensor("vlt", [128, B // 128], BF16, kind="ExternalInput")
    lnb = nc.dram_tensor("lnb", [1, 1], F32, kind="ExternalInput")
    out_d = nc.dram_tensor("out", [1, BL], F32, kind="ExternalOutput")

    NJ = B // 128  # 32 j-tiles of the score matrix (columns of z = rows of zT)
    with tile.TileContext(nc) as tc, ExitStack() as ctx:
        pool = ctx.enter_context(tc.tile_pool(name="p2", bufs=1))
        stream = ctx.enter_context(tc.tile_pool(name="p2s", bufs=8))
        zps = ctx.enter_context(tc.tile_pool(name="zps", bufs=2, space="PSUM"))
        accps = ctx.enter_context(tc.tile_pool(name="accps", bufs=1, space="PSUM"))

        qt_sb = pool.tile([128, 4, BL], ADT, tag="qt")
        nc.sync.dma_start(out=qt_sb, in_=qt[:, :, :])
        vlt_sb = pool.tile([128, B // 128], BF16, tag="vlt")
        nc.sync.dma_start(out=vlt_sb, in_=vlt[:, :])
        lnb_sb = pool.tile([1, 1], F32, tag="lnb")
        nc.sync.dma_start(out=lnb_sb, in_=lnb[:, :])
        ones = pool.tile([128, 1], BF16, tag="ones")
        nc.vector.memset(ones, 1.0)

        sums_ps = accps.tile([1, BL], F32, tag="sums")
        r_ps = accps.tile([1, BL], F32, tag="racc")

        for t in range(NJ):
            r, q4 = t // 4, t % 4
            kblk = stream.tile([128, 4, 128], ADT, tag="kblk")
            nc.sync.dma_start(out=kblk, in_=kb[t, :, :, :])
            zp = zps.tile([128, BL], F32, tag="zp")
            if FP8:
                for k in (0, 2):
                    nc.tensor.matmul(zp, kblk[:, k:k + 2, :],
                                     qt_sb[:, k:k + 2, :], start=(k == 0),
                                     stop=(k == 2), perf_mode=DR)
            else:
                for k in range(4):
                    nc.tensor.matmul(zp, kblk[:, k, :], qt_sb[:, k, :],
                                     start=(k == 0), stop=(k == 3))
            ex = stream.tile([128, BL], BF16, tag="ex")
            nc.scalar.activation(ex, zp, AF.Exp,
                                 scale=float(1.0 / (QKS * QKS * np.sqrt(H))))
            nc.tensor.matmul(sums_ps, ones, ex, start=(t == 0),
                             stop=(t == NJ - 1))
            nc.tensor.matmul(r_ps, vlt_sb[:, t:t + 1], ex, start=(t == 0),
                             stop=(t == NJ - 1))

        recip = pool.tile([1, BL], F32, tag="recip")
        nc.vector.reciprocal(recip, sums_ps)
        prod = pool.tile([1, BL], F32, tag="prod")
        nc.vector.tensor_mul(prod, r_ps, recip)
        osb = pool.tile([1, BL], F32, tag="osb")
        nc.scalar.activation(osb, prod, AF.Sigmoid, bias=lnb_sb[0:1, 0:1])
        nc.sync.dma_start(out=out_d[:, :], in_=osb)

    nc.compile()
    return nc


def _prep_consts(inp):
    """Host-side weight packing (shared by all cores)."""
    f64 = np.float64
    w1, b1 = inp["rcnn_w1"].astype(f64), inp["rcnn_b1"].astype(f64)
    w2, b2 = inp["rcnn_w2"].astype(f64), inp["rcnn_b2"].astype(f64)
    w3, b3 = inp["rcnn_w3"].astype(f64), inp["rcnn_b3"].astype(f64)
    # fold conv1 (1x1, D->16) into conv2 (3-tap, 16->32):
    # w12[s,d,k,c2] = sum_c w2[s,c2,c,k] * w1[s,c,d];  b12[s,c2] folds b1.
    w12 = np.einsum("sack,scd->sdka", w2, w1)          # [S, 128, 3, 32]
    b12 = b2 + np.einsum("sack,sc->sa", w2, b1)        # [S, 32]
    # conv2's (folded) bias commutes past the maxpool into conv4's bias:
    # b3eff[s,a] = b3[s,a] + sum_{c,k} w3[s,a,c,k] * b12eff[s,c]
    b3eff = b3 + np.einsum("sack,sc->sa", w3, b12)
    consts = {
        "w12": np.ascontiguousarray(w12.transpose(1, 0, 2, 3) * WS).astype(nADT),
        "b12": np.ascontiguousarray(b12.T).astype(np.float32),
        "w3d": np.ascontiguousarray(w3.transpose(2, 0, 3, 1) / WS).astype(nbf16),
        "b3d": np.ascontiguousarray(b3eff.T * HS).astype(np.float32),
    }

    def pack_gate_T(wT):   # [in_f, 2048] -> [128, in_f//128, 2048]
        nk = wT.shape[0] // 128
        return np.ascontiguousarray(
            (wT * WS).reshape(nk, 128, -1).transpose(1, 0, 2)).astype(nADT)

    def pack_sq(wT):       # [512, N] -> [128, 4, N]
        return np.ascontiguousarray(
            (wT * WS).reshape(4, 128, -1).transpose(1, 0, 2)).astype(nADT)

    dec_wih = inp["dec_wih"].astype(np.float32)
    consts.update({
        "wih": (inp["enc_wih"].T * WS).astype(nADT),
        "whh": pack_gate_T(inp["enc_whh"].T.astype(np.float32)),
        "bge": np.ascontiguousarray(
            (inp["enc_bih"] + inp["enc_bhh"]).reshape(16, 128).T
        ).astype(np.float32),
        "dxw": pack_gate_T(dec_wih[:, :H].T),
        "dwy": (dec_wih[:, H] * WS).reshape(1, -1).astype(nADT),
        "dhw": pack_gate_T(inp["dec_whh"].T.astype(np.float32)),
        "bgd": np.ascontiguousarray(
            (inp["dec_bih"] + inp["dec_bhh"]).reshape(16, 128).T
        ).astype(np.float32),
        "wvl": np.ascontiguousarray(
            (inp["wv"].astype(f64).T @ inp["ln_w"].astype(f64).reshape(H)
             * WS).reshape(4, 128).T).astype(nADT),
        "wqt": pack_sq(inp["wq"].T.astype(np.float32)),
        "wkt": pack_sq(inp["wk"].T.astype(np.float32)),
    })
    lnw = np.ascontiguousarray(
        inp["ln_w"].reshape(H).reshape(4, 128).T).astype(nbf16)  # [128, 4]
    lnb = inp["ln_b"].reshape(1, 1).astype(np.float32)
    return consts, lnw, lnb


def kernel(**inputs):
    if not TRACE:
        # NTFF tracing needs antenv.axon_hooks, absent in this container;
        # make sure an inherited BASS_TRACE=1 can't crash the run.
        os.environ["BASS_NEVER_TRACE"] = "1"
    inputs = {k: np.asarray(v) for k, v in inputs.items()}
    if "p1" not in _CACHE:
        _CACHE["p1"] = _build_phase1()
    if "p2" not in _CACHE:
        _CACHE["p2"] = _build_phase2()
    p1, p2 = _CACHE["p1"], _CACHE["p2"]

    consts, lnw, lnb = _prep_consts(inputs)
    x = inputs["x"].astype(nADT)
    y = inputs["y"].astype(np.float32)

    in_maps1 = []
    for c in range(NCORES):
        b0 = c * BL
        ysel_np = np.ascontiguousarray(
            y[b0:b0 + BL][:, IDX].T * HS).reshape(1, TP * BL).astype(nADT)
        xt = x[b0:b0 + BL].transpose(2, 1, 0)          # [D, T, BL]
        xc = np.stack([xt[:, :, i * BC:(i + 1) * BC]
                       for i in range(BL // BC)])        # [4, D, T, BC]
        m = {"x": np.ascontiguousarray(xc), "ysel": ysel_np}
        m.update(consts)
        in_maps1.append(m)

    r1 = run_bass_kernel_spmd(p1, in_maps1, core_ids=list(range(NCORES)),
                              trace=TRACE)
    LAST_EXEC_NS[0] = r1.exec_time_ns

    kb = np.concatenate([r1.results[c]["kt"] for c in range(NCORES)], axis=0)
    # [512r + 128k + p, 128q + j] -> [t=(r,q), p, k, j], contiguous per j-tile
    kb = np.ascontiguousarray(
        kb.reshape(NCORES, 4, 128, 4, 128).transpose(0, 3, 2, 1, 4)
        .reshape(B // 128, 128, 4, 128))
    vl_full = np.concatenate(
        [r1.results[c]["vl"].T.reshape(BL) for c in range(NCORES)])
    vlt_np = np.ascontiguousarray(
        vl_full.reshape(B // 128, 128).T).astype(nbf16)
    in_maps2 = [
        {"qt": np.ascontiguousarray(
            r1.results[c]["qt"].reshape(4, 128, BL).transpose(1, 0, 2)),
         "kb": kb, "vlt": vlt_np, "lnb": lnb}
        for c in range(NCORES)
    ]
    r2 = run_bass_kernel_spmd(p2, in_maps2, core_ids=list(range(NCORES)),
                              trace=TRACE)
    LAST_EXEC_NS[1] = r2.exec_time_ns

    out = np.concatenate([r2.results[c]["out"][0] for c in range(NCORES)])
    return out.astype(np.float32)



# revision 16
# speedup vs baseline: 1.4230x; 1.4230x over previous
"""DA-RNN + batch self-attention Trainium2 kernel (8 NeuronCores, SPMD).

Strategy: data-parallel over batch (B=4096 -> 512/core) for CNN + encoder LSTM +
decoder LSTM + q/k/v projections (phase 1).  Host gathers k/v across cores, then
phase 2 computes the BxB softmax attention with score-matrix rows sharded
across cores (each core holds full softmax rows for its 512 queries).

Engine-balance design (cost-model driven):
 - every fp8 matmul runs in DoubleRow mode at psum partition 0; single-k-tile
   passes pair their real rows with a zero/bias slot, and conv matmuls pair
   two (branch, position) outputs per pass through the slot dimension
 - LSTM biases enter through matmul pad slots (constant-1 moving rows), so
   gate activations need no per-m-tile bias and merge into 4-bank-wide ops
 - encoder/decoder steps interleave (enc t || dec t-1) so each LSTM's
   elementwise tail hides under the other's matmuls/activations
 - cell state is bf16 in SBUF (2x DVE rate); conv12 maxpool goes through an
   Activation-engine psum->sbuf copy + strided tensor-tensor max on DVE
 - h3 is branch-packed in partitions with per-branch position shifts so conv3
   is a single block-diagonal 128-partition DR matmul per output position,
   its bias folded into the pad slot and maxpool2 writing featT directly

Self-contained: hardcodes all shapes; takes the full unsharded inputs.
"""

import os
import numpy as np
import ml_dtypes
from contextlib import ExitStack
from itertools import groupby

import concourse.mybir as mybir
import concourse.tile as tile
from concourse import bacc
from concourse.bass_utils import run_bass_kernel_spmd

F32 = mybir.dt.float32
BF16 = mybir.dt.bfloat16
FP8E4 = mybir.dt.float8e4
DR = mybir.MatmulPerfMode.DoubleRow
AF = mybir.ActivationFunctionType
MUL = mybir.AluOpType.mult
ADD = mybir.AluOpType.add
MAX = mybir.AluOpType.max
nbf16 = ml_dtypes.bfloat16
nfp8 = ml_dtypes.float8_e4m3

B, T, D, H, S = 4096, 45, 128, 512, 4
NCORES = 8
BL = B // NCORES          # 512 batch rows per core
BC = 128                  # CNN batch chunk
TP = 9                    # downsampled sequence length
IDX = list(range(T - 1, 0, -(T // TP)))[::-1]   # [4,9,...,44]
NL4 = [18, 8, 4, 2]       # conv3 output positions consumed per branch
NLO = [40, 20, 12, 8]     # conv12 positions needed per branch
T0 = [0, 5, 7, 8]         # featT start index per branch
H3PAD = 20                # h3 pad position (constant 1.0, bias carrier)
FPAD = TP                 # featT pad position (constant 1.0, bias carrier)

WS = 16.0                 # weight prescale
HS = 8.0                  # hidden/feat/y prescale
K3 = 8.0                  # extra conv3/featT scale (better fp8 resolution)
SC = 1.0 / (WS * HS)      # psum -> true preactivation scale
QKS = 4.0                 # extra prescale on stored q/k

# exec times of the two launches from the most recent kernel() call (ns or None)
LAST_EXEC_NS = [None, None]
TRACE = False
_CACHE = {}


def _conv12_plan():
    """Pair-matmul emission plan for conv12.

    psum tile layout: A-tiles [64, 8, BC], global position q = 8g+sub with
    branch 0 at rows 0-31 (conv pos q) and branch 1 at rows 32-63 (conv pos
    q-20, valid q>=20).  B-tile [64, 12, BC]: branch 2 rows 0-31 (pos v),
    branch 3 rows 32-63 (pos v-4, valid v>=4).  The position shifts make
    pooled outputs land at matching h3 positions per branch.

    Returns (vkeys, tiles): vkeys name the stationary-weight variants
    (rebuilt identically on the host); tiles = list of
    (kind, g, nsub, passes), passes = (sub, variant_idx, x_lo, x_step).
    """
    vmap, vkeys = {}, []

    def vi(key):
        if key not in vmap:
            vmap[key] = len(vkeys)
            vkeys.append(key)
        return vmap[key]

    def passes_for(sub, sa, pa, sb=None, pb=None):
        out = []
        if sb is None:
            st = sa + 1
            out.append((sub, vi(("s", sa, 0)), pa * st, st))
            out.append((sub, vi(("s", sa, 2)), (pa + 2) * st, 1))
        else:
            for k in range(3):
                p0 = (pa + k) * (sa + 1)
                p1 = (pb + k) * (sb + 1)
                if p0 < p1:
                    out.append((sub, vi(("p", sa, sb, k, 0)), p0, p1 - p0))
                elif p0 > p1:
                    out.append((sub, vi(("p", sa, sb, k, 1)), p1, p0 - p1))
                else:
                    out.append((sub, vi(("p", sa, sb, k, 2)), p0, 1))
        return out

    tiles = []
    for g in range(5):
        pl = []
        for sub in range(8):
            q = 8 * g + sub
            if q < 20:
                pl += passes_for(sub, 0, q)
            else:
                pl += passes_for(sub, 0, q, 1, q - 20)
        tiles.append(("A", g, 8, pl))
    pl = []
    for sub in range(8):
        if sub < 4:
            pl += passes_for(sub, 2, sub)
        else:
            pl += passes_for(sub, 2, sub, 3, sub - 4)
    tiles.append(("B", 0, 8, pl))
    pl = []
    for sub in range(8, 12):
        pl += passes_for(sub - 8, 2, sub, 3, sub - 4)
    tiles.append(("B2", 0, 4, pl))
    return vkeys, tiles


_VKEYS, _C12TILES = _conv12_plan()
NV12 = len(_VKEYS)


def _build_phase1():
    nc = bacc.Bacc("TRN2", target_bir_lowering=False, debug=False,
                   num_devices=NCORES)
    x = nc.dram_tensor("x", [BL // BC, D, T + 1, BC], FP8E4,
                       kind="ExternalInput")
    ydr = nc.dram_tensor("ydr", [1, 2, TP * BL], FP8E4, kind="ExternalInput")
    w12 = nc.dram_tensor("w12", [128, 2, NV12, 64], FP8E4,
                         kind="ExternalInput")
    w3p = nc.dram_tensor("w3p", [128, 2, 5, 128], FP8E4, kind="ExternalInput")
    wihp = nc.dram_tensor("wihp", [128, 2, 16 * 128], FP8E4,
                          kind="ExternalInput")
    whhp = nc.dram_tensor("whhp", [128, 4, 16 * 128], FP8E4,
                          kind="ExternalInput")
    dxwp = nc.dram_tensor("dxwp", [128, 4, 16 * 128], FP8E4,
                          kind="ExternalInput")
    ydrw = nc.dram_tensor("ydrw", [1, 2, 16 * 128], FP8E4,
                          kind="ExternalInput")
    dhwp = nc.dram_tensor("dhwp", [128, 4, 16 * 128], FP8E4,
                          kind="ExternalInput")
    wqt = nc.dram_tensor("wqt", [128, 4, H], FP8E4, kind="ExternalInput")
    wkt = nc.dram_tensor("wkt", [128, 4, H], FP8E4, kind="ExternalInput")
    wvl = nc.dram_tensor("wvl", [128, 4], FP8E4, kind="ExternalInput")
    qt_d = nc.dram_tensor("qt", [4 * 128, BL], FP8E4, kind="ExternalOutput")
    kt_d = nc.dram_tensor("kt", [4 * 128, BL], FP8E4, kind="ExternalOutput")
    vl_d = nc.dram_tensor("vl", [128, 4], BF16, kind="ExternalOutput")

    with tile.TileContext(nc) as tc, ExitStack() as ctx:
        wpool = ctx.enter_context(tc.tile_pool(name="wpool", bufs=1))
        state = ctx.enter_context(tc.tile_pool(name="state", bufs=1))

        # CNN weights first (conv starts as soon as x chunk 0 lands)
        w12_sb = wpool.tile([128, 2, NV12, 64], FP8E4, tag="w12",
                            name="w12_sb")
        nc.sync.dma_start(out=w12_sb, in_=w12[:, :, :, :])
        w3_sb = wpool.tile([128, 2, 5, 128], FP8E4, tag="w3", name="w3_sb")
        nc.sync.dma_start(out=w3_sb, in_=w3p[:, :, :, :])

        featT = state.tile([128, TP + 1, BL], FP8E4, tag="featT", name="featT")
        nc.vector.memset(featT, 0.0)
        nc.vector.memset(featT[:, FPAD, :], 1.0)
        hencT = state.tile([128, TP, 4, BL], FP8E4, tag="hencT", name="hencT")

        cnnx = ctx.enter_context(tc.tile_pool(name="cnnx", bufs=1))
        xts = []
        for ci in range(BL // BC):
            xT = cnnx.tile([128, T + 1, BC], FP8E4, tag=f"xT{ci}",
                           name=f"xT{ci}")
            nc.sync.dma_start(out=xT, in_=x[ci, :, :, :])
            xts.append(xT)

        # LSTM weights (DMA overlaps the CNN)
        wihp_sb = wpool.tile([128, 2, 16 * 128], FP8E4, tag="wihp",
                             name="wihp_sb")
        nc.sync.dma_start(out=wihp_sb, in_=wihp[:, :, :])
        whhp_sb = wpool.tile([128, 4, 16 * 128], FP8E4, tag="whhp",
                             name="whhp_sb")
        nc.sync.dma_start(out=whhp_sb, in_=whhp[:, :, :])
        ydr_sb = wpool.tile([1, 2, TP * BL], FP8E4, tag="ydr", name="ydr_sb")
        nc.sync.dma_start(out=ydr_sb, in_=ydr[:, :, :])
        dxwp_sb = wpool.tile([128, 4, 16 * 128], FP8E4, tag="dxwp",
                             name="dxwp_sb")
        nc.sync.dma_start(out=dxwp_sb, in_=dxwp[:, :, :])
        ydrw_sb = wpool.tile([1, 2, 16 * 128], FP8E4, tag="ydrw",
                             name="ydrw_sb")
        nc.sync.dma_start(out=ydrw_sb, in_=ydrw[:, :, :])
        dhwp_sb = wpool.tile([128, 4, 16 * 128], FP8E4, tag="dhwp",
                             name="dhwp_sb")
        nc.sync.dma_start(out=dhwp_sb, in_=dhwp[:, :, :])
        wq_sb = wpool.tile([128, 4, H], FP8E4, tag="wq", name="wq_sb")
        nc.sync.dma_start(out=wq_sb, in_=wqt[:, :, :])
        wk_sb = wpool.tile([128, 4, H], FP8E4, tag="wk", name="wk_sb")
        nc.sync.dma_start(out=wk_sb, in_=wkt[:, :, :])
        wvl_sb = wpool.tile([128, 4], FP8E4, tag="wvl", name="wvl_sb")
        nc.sync.dma_start(out=wvl_sb, in_=wvl[:, :])

        # ---------------- CNN downsampling ----------------
        h3s = []
        for ci in range(BL // BC):
            h3 = state.tile([128, H3PAD + 1, BC], FP8E4, tag=f"h3{ci}",
                            name=f"h3_{ci}")
            nc.gpsimd.memset(h3, 0.0)
            nc.gpsimd.memset(h3[:, H3PAD, :], 1.0)
            h3s.append(h3)

        with (
            tc.tile_pool(name="cpsA", bufs=2, space="PSUM") as cpsA,
            tc.tile_pool(name="cpsB", bufs=1, space="PSUM") as cpsB,
            tc.tile_pool(name="cps3", bufs=1, space="PSUM") as cps3,
            tc.tile_pool(name="hcopy", bufs=3) as hcopy,
        ):
            def ttmax(out, in0, in1):
                nc.vector.tensor_tensor(out, in0, in1, MAX)

            for ci in range(BL // BC):
                xT = xts[ci]
                h3 = h3s[ci]
                cc = slice(ci * BC, (ci + 1) * BC)
                for (kind, g, nsub, passes) in _C12TILES:
                    pool_, tg = (cpsA, "cA") if kind == "A" else (cpsB, "cB")
                    ps = pool_.tile([64, 8, BC], F32, tag=tg,
                                    name=f"c12_{ci}_{kind}{g}")
                    for sub, grp in groupby(passes, key=lambda e: e[0]):
                        grp = list(grp)
                        for idx, (_, v, plo, step) in enumerate(grp):
                            nout = 64 if _VKEYS[v][0] == "p" else 32
                            nc.tensor.matmul(
                                ps[0:nout, sub, :], w12_sb[:, :, v, 0:nout],
                                xT[:, plo:plo + step + 1:step, :],
                                start=(idx == 0), stop=(idx == len(grp) - 1),
                                perf_mode=DR)
                    hc = hcopy.tile([64, nsub, BC], BF16,
                                    tag=f"hc{kind}", name=f"hc_{ci}_{kind}{g}")
                    nc.scalar.activation(hc, ps[:, 0:nsub, :], AF.Identity)
                    n2 = nsub
                    if kind == "A":
                        if g <= 1:
                            ttmax(h3[0:32, 4 * g:4 * g + 4, :],
                                  hc[0:32, 0:n2:2, :], hc[0:32, 1:n2:2, :])
                        elif g == 2:
                            ttmax(h3[0:32, 8:12, :],
                                  hc[0:32, 0:n2:2, :], hc[0:32, 1:n2:2, :])
                            ttmax(h3[32:64, 10:12, :],
                                  hc[32:64, 4:n2:2, :], hc[32:64, 5:n2:2, :])
                        else:
                            ttmax(h3[0:64, 4 * g:4 * g + 4, :],
                                  hc[0:64, 0:n2:2, :], hc[0:64, 1:n2:2, :])
                    elif kind == "B":
                        ttmax(h3[64:96, 14:18, :],
                              hc[0:32, 0:n2:2, :], hc[0:32, 1:n2:2, :])
                        ttmax(h3[96:128, 16:18, :],
                              hc[32:64, 4:n2:2, :], hc[32:64, 5:n2:2, :])
                    else:
                        ttmax(h3[64:96, 18:20, :],
                              hc[0:32, 0:n2:2, :], hc[0:32, 1:n2:2, :])
                        ttmax(h3[96:128, 18:20, :],
                              hc[32:64, 0:n2:2, :], hc[32:64, 1:n2:2, :])
                # conv3 + maxpool2 (writes featT directly)
                for blk, (o0, no) in enumerate(((0, 8), (8, 8), (16, 2))):
                    ps = cps3.tile([128, 8, BC], F32, tag="c3",
                                   name=f"c3_{ci}_{blk}")
                    for sub in range(no):
                        o = o0 + sub
                        rv = 1 + (o >= 10) + (o >= 14) + (o >= 16)
                        nc.tensor.matmul(ps[:, sub, :], w3_sb[:, :, 0, :],
                                         h3[:, o:o + 2, :], start=True,
                                         stop=False, perf_mode=DR)
                        st = H3PAD - o - 2
                        nc.tensor.matmul(ps[:, sub, :], w3_sb[:, :, rv, :],
                                         h3[:, o + 2:H3PAD + 1:st, :],
                                         start=False, stop=True, perf_mode=DR)
                    pv = ps[:, 0:no, :].rearrange("c (l two) b -> c l b two",
                                                  two=2)
                    nc.vector.tensor_reduce(
                        featT[:, o0 // 2:(o0 + no) // 2, cc], pv,
                        mybir.AxisListType.X, MAX)

        # ---------------- interleaved encoder/decoder ----------------
        gpsum = ctx.enter_context(tc.tile_pool(name="gpsum", bufs=2,
                                               space="PSUM"))
        gact = ctx.enter_context(tc.tile_pool(name="gact", bufs=1))
        cpool = ctx.enter_context(tc.tile_pool(name="cpool", bufs=2))
        ttmp = ctx.enter_context(tc.tile_pool(name="ttmp", bufs=3))
        tchp = ctx.enter_context(tc.tile_pool(name="tchp", bufs=2))
        hdp = ctx.enter_context(tc.tile_pool(name="hdp", bufs=2))

        def emit_mms(kind, t, htp, g, ps, rhs_h):
            for j in range(2):
                ht = 2 * htp + j
                cs = slice((4 * g + ht) * 128, (4 * g + ht + 1) * 128)
                if kind == "e":
                    nc.tensor.matmul(
                        ps[:, ht, :], wihp_sb[:, :, cs],
                        featT[:, t:FPAD + 1:FPAD - t, :],
                        start=True, stop=(rhs_h is None), perf_mode=DR)
                else:
                    for k in (0, 2):
                        nc.tensor.matmul(
                            ps[:, ht, :], dxwp_sb[:, k:k + 2, cs],
                            hencT[:, t, k:k + 2, :], start=(k == 0),
                            stop=False, perf_mode=DR)
                    nc.tensor.matmul(
                        ps[:, ht, :], ydrw_sb[:, :, cs],
                        ydr_sb[:, :, t * BL:(t + 1) * BL],
                        start=False, stop=(rhs_h is None), perf_mode=DR)
                if rhs_h is not None:
                    hw_sb = whhp_sb if kind == "e" else dhwp_sb
                    for k in (0, 2):
                        nc.tensor.matmul(
                            ps[:, ht, :], hw_sb[:, k:k + 2, cs],
                            rhs_h[:, k:k + 2, :], start=False,
                            stop=(k == 2), perf_mode=DR)

        def emit_tail(kind, t, sl, c_prev, c_new, acts, h_out):
            if t == 0:
                nc.vector.tensor_tensor(c_new[:, sl, :], acts[0][:, sl, :],
                                        acts[2][:, sl, :], MUL)
            else:
                n = sl.stop - sl.start
                t1 = ttmp.tile([128, n, BL], BF16, tag=f"tt{n}",
                               name=f"t1_{kind}_{t}_{sl.start}")
                nc.vector.tensor_tensor(t1, acts[1][:, sl, :],
                                        c_prev[:, sl, :], MUL)
                t2 = ttmp.tile([128, n, BL], BF16, tag=f"tt{n}",
                               name=f"t2_{kind}_{t}_{sl.start}")
                nc.vector.tensor_tensor(t2, acts[0][:, sl, :],
                                        acts[2][:, sl, :], MUL)
                nc.vector.tensor_tensor(c_new[:, sl, :], t1, t2, ADD)
            n = sl.stop - sl.start
            tch = tchp.tile([128, n, BL], BF16, tag=f"tch{n}",
                            name=f"tch_{kind}_{t}_{sl.start}")
            nc.scalar.activation(tch, c_new[:, sl, :], AF.Tanh)
            nc.vector.scalar_tensor_tensor(h_out[:, sl, :], acts[3][:, sl, :],
                                           HS, tch, MUL, MUL)

        def emit_step(kind, t, rhs_h, c_prev, c_new, h_out, split=False):
            gts = (0, 2, 3) if t == 0 else (0, 1, 2, 3)
            acts = {g: gact.tile([128, 4, BL], BF16, tag=f"{kind}a{g}",
                                 name=f"a_{kind}_{t}_{g}")
                    for g in gts}
            if not split:
                for g in gts:
                    ps = gpsum.tile([128, 4, BL], F32, tag="gps",
                                    name=f"gps_{kind}_{t}_{g}")
                    for htp in (0, 1):
                        emit_mms(kind, t, htp, g, ps, rhs_h)
                    nc.scalar.activation(acts[g], ps,
                                         AF.Tanh if g == 2 else AF.Sigmoid,
                                         scale=SC)
                emit_tail(kind, t, slice(0, 4), c_prev, c_new, acts, h_out)
            else:
                # finer-grained finale: per-gtype acts split in ht halves so
                # the serial tail chain of the last step is shorter
                for g in gts:
                    ps = gpsum.tile([128, 4, BL], F32, tag="gps",
                                    name=f"gps_{kind}_{t}_{g}")
                    for htp in (0, 1):
                        emit_mms(kind, t, htp, g, ps, rhs_h)
                        nc.scalar.activation(
                            acts[g][:, 2 * htp:2 * htp + 2, :],
                            ps[:, 2 * htp:2 * htp + 2, :],
                            AF.Tanh if g == 2 else AF.Sigmoid, scale=SC)
                for htp in (0, 1):
                    emit_tail(kind, t, slice(2 * htp, 2 * htp + 2),
                              c_prev, c_new, acts, h_out)

        ce_prev = cd_prev = None
        hd_prev = None
        for t in range(TP + 1):
            if t < TP:
                ce_new = cpool.tile([128, 4, BL], BF16, tag="ce",
                                    name=f"ce_{t}")
                emit_step("e", t, None if t == 0 else hencT[:, t - 1, :, :],
                          ce_prev, ce_new, hencT[:, t, :, :])
                ce_prev = ce_new
            if t >= 1:
                td = t - 1
                cd_new = cpool.tile([128, 4, BL], BF16, tag="cd",
                                    name=f"cd_{td}")
                hd_new = hdp.tile([128, 4, BL], FP8E4, tag="hd",
                                  name=f"hd_{td}")
                emit_step("d", td, hd_prev, cd_prev, cd_new, hd_new,
                          split=(td == TP - 1))
                cd_prev, hd_prev = cd_new, hd_new

        # ---------------- q/k/v projections ----------------
        qout = state.tile([128, 4, BL], FP8E4, tag="qout", name="qout")
        kout = state.tile([128, 4, BL], FP8E4, tag="kout", name="kout")
        vlout = state.tile([128, 4], BF16, tag="vlout", name="vlout")
        for w_sb, osb, eng in ((wq_sb, qout, "act"), (wk_sb, kout, "dve")):
            ps = gpsum.tile([128, 4, BL], F32, tag="gps", name=f"qk_{eng}")
            for mh in range(4):
                for k in (0, 2):
                    nc.tensor.matmul(
                        ps[:, mh, :],
                        w_sb[:, k:k + 2, mh * 128:(mh + 1) * 128],
                        hd_prev[:, k:k + 2, :], start=(k == 0),
                        stop=(k == 2), perf_mode=DR)
            if eng == "act":
                nc.scalar.activation(osb, ps, AF.Identity, scale=SC * QKS)
            else:
                nc.vector.tensor_scalar_mul(osb, ps, SC * QKS)
        vlps = gpsum.tile([128, 4, BL], F32, tag="gps", name="vlps")
        for mi in range(4):
            for k in range(4):
                nc.tensor.matmul(vlps[:, 0, mi:mi + 1],
                                 hd_prev[:, k, mi * 128:(mi + 1) * 128],
                                 wvl_sb[:, k:k + 1], start=(k == 0),
                                 stop=(k == 3))
        nc.vector.tensor_scalar_mul(vlout[:, :], vlps[:, 0, 0:4], SC)
        nc.sync.dma_start(out=qt_d.rearrange("(k p) i -> p k i", p=128),
                          in_=qout)
        nc.sync.dma_start(out=kt_d.rearrange("(k p) i -> p k i", p=128),
                          in_=kout)
        nc.sync.dma_start(out=vl_d[:, :], in_=vlout)

    nc.compile()
    return nc


def _build_phase2():
    nc = bacc.Bacc("TRN2", target_bir_lowering=False, debug=False,
                   num_devices=NCORES)
    qt = nc.dram_tensor("qt", [128, 4, BL], FP8E4, kind="ExternalInput")
    kb = nc.dram_tensor("kb", [128, B // 128, 4, 128], FP8E4,
                        kind="ExternalInput")
    sv = nc.dram_tensor("sv", [128, B // 128, 33], BF16, kind="ExternalInput")
    lnb = nc.dram_tensor("lnb", [1, 1], F32, kind="ExternalInput")
    out_d = nc.dram_tensor("out", [1, BL], F32, kind="ExternalOutput")

    NJ = B // 128  # 32 j-tiles of the score matrix
    with tile.TileContext(nc) as tc, ExitStack() as ctx:
        pool = ctx.enter_context(tc.tile_pool(name="p2", bufs=1))
        expool = ctx.enter_context(tc.tile_pool(name="p2e", bufs=3))
        zps = ctx.enter_context(tc.tile_pool(name="zps", bufs=3, space="PSUM"))
        srp = ctx.enter_context(tc.tile_pool(name="srp", bufs=1, space="PSUM"))

        qt_sb = pool.tile([128, 4, BL], FP8E4, tag="qt", name="qt_sb")
        nc.sync.dma_start(out=qt_sb, in_=qt[:, :, :])
        sv_sb = pool.tile([128, NJ, 33], BF16, tag="sv", name="sv_sb")
        nc.sync.dma_start(out=sv_sb, in_=sv[:, :, :])
        lnb_sb = pool.tile([1, 1], F32, tag="lnb", name="lnb_sb")
        nc.sync.dma_start(out=lnb_sb, in_=lnb[:, :])
        kb_sb = pool.tile([128, NJ, 4, 128], FP8E4, tag="kb", name="kb_sb")
        for c in range(8):
            nc.sync.dma_start(out=kb_sb[:, 4 * c:4 * (c + 1), :, :],
                              in_=kb[:, 4 * c:4 * (c + 1), :, :])

        # software-pipelined: z-matmuls for pair pi+1 are emitted before the
        # sums/r matmuls of pair pi, so the in-order PE queue never waits on
        # the exp that feeds them
        NP2 = NJ // 2
        sr_ps = srp.tile([33, BL], F32, tag="sr", name="sr_ps")
        exs = [None] * NP2

        def emit_z(pi):
            zp = zps.tile([128, 2, BL], F32, tag="zp", name=f"zp_{pi}")
            for j in range(2):
                tt = 2 * pi + j
                for k in (0, 2):
                    nc.tensor.matmul(zp[:, j, :], kb_sb[:, tt, k:k + 2, :],
                                     qt_sb[:, k:k + 2, :], start=(k == 0),
                                     stop=(k == 2), perf_mode=DR)
            ex = expool.tile([128, 2, BL], BF16, tag="ex", name=f"ex_{pi}")
            nc.scalar.activation(ex, zp, AF.Exp,
                                 scale=float(1.0 / (QKS * QKS * np.sqrt(H))))
            exs[pi] = ex

        def emit_sr(pi):
            for j in range(2):
                nc.tensor.matmul(sr_ps, sv_sb[:, 2 * pi + j, :],
                                 exs[pi][:, j, :],
                                 start=(pi == 0 and j == 0),
                                 stop=(pi == NP2 - 1 and j == 1))

        emit_z(0)
        emit_z(1)
        for pi in range(NP2):
            if pi + 2 < NP2:
                emit_z(pi + 2)
            emit_sr(pi)

        recip = pool.tile([1, BL], F32, tag="recip", name="recip")
        nc.vector.reciprocal(recip, sr_ps[0:1, :])
        prod = pool.tile([1, BL], F32, tag="prod", name="prod")
        nc.vector.tensor_tensor(prod, sr_ps[32:33, :], recip, MUL)
        osb = pool.tile([1, BL], F32, tag="osb", name="osb")
        nc.scalar.activation(osb, prod, AF.Sigmoid, bias=lnb_sb[0:1, 0:1])
        nc.sync.dma_start(out=out_d[:, :], in_=osb)

    nc.compile()
    return nc


def _prep_consts(inp):
    """Host-side weight packing (shared by all cores)."""
    f64 = np.float64
    w1, b1 = inp["rcnn_w1"].astype(f64), inp["rcnn_b1"].astype(f64)
    w2, b2 = inp["rcnn_w2"].astype(f64), inp["rcnn_b2"].astype(f64)
    w3, b3 = inp["rcnn_w3"].astype(f64), inp["rcnn_b3"].astype(f64)
    # fold conv1 (1x1, D->16) into conv2 (3-tap, 16->32):
    w12 = np.einsum("sack,scd->sdka", w2, w1)          # [S, 128, 3, 32]
    b12 = b2 + np.einsum("sack,sc->sa", w2, b1)        # [S, 32]
    # conv2's (folded) bias commutes past the maxpool into conv4's bias
    b3eff = b3 + np.einsum("sack,sc->sa", w3, b12)

    w12b = np.zeros((128, 2, NV12, 64), np.float32)
    for i, key in enumerate(_VKEYS):
        if key[0] == "s":
            _, s, k0 = key
            if k0 == 0:
                w12b[:, 0, i, 0:32] = w12[s, :, 0, :] * WS
                w12b[:, 1, i, 0:32] = w12[s, :, 1, :] * WS
            else:
                w12b[:, 0, i, 0:32] = w12[s, :, 2, :] * WS
        else:
            _, sa, sb, k, order = key
            wa = w12[sa, :, k, :] * WS
            wb = w12[sb, :, k, :] * WS
            if order == 0:
                w12b[:, 0, i, 0:32] = wa
                w12b[:, 1, i, 32:64] = wb
            elif order == 1:
                w12b[:, 0, i, 32:64] = wb
                w12b[:, 1, i, 0:32] = wa
            else:
                w12b[:, 0, i, 0:32] = wa
                w12b[:, 0, i, 32:64] = wb

    # conv3 block-diagonal stationaries: v0 = taps (0,1); v1..v4 = tap2 +
    # bias covering the first rv branches (invalid positions get no bias)
    w3b = np.zeros((128, 2, 5, 128), np.float32)
    for s in range(S):
        r0 = 32 * s
        for k in (0, 1):
            w3b[r0:r0 + 32, k, 0, r0:r0 + 32] = \
                w3[s].transpose(1, 0, 2)[:, :, k] * (HS * K3 / WS)
        for rv in range(1, 5):
            w3b[r0:r0 + 32, 0, rv, r0:r0 + 32] = \
                w3[s].transpose(1, 0, 2)[:, :, 2] * (HS * K3 / WS)
            if s < rv:
                w3b[r0, 1, rv, r0:r0 + 32] = b3eff[s] * (HS * K3)

    def pack_gate_T(wT):   # [in_f, 2048] -> [128, in_f//128, 2048]
        nk = wT.shape[0] // 128
        return np.ascontiguousarray(
            (wT * WS).reshape(nk, 128, -1).transpose(1, 0, 2)).astype(nfp8)

    def pack_sq(wT):       # [512, N] -> [128, 4, N]
        return np.ascontiguousarray(
            (wT * WS).reshape(4, 128, -1).transpose(1, 0, 2)).astype(nfp8)

    wihp = np.zeros((128, 2, 16 * 128), np.float32)
    wihp[:, 0, :] = inp["enc_wih"].T.astype(np.float32) * (WS / K3)
    wihp[0, 1, :] = (inp["enc_bih"] + inp["enc_bhh"]).astype(np.float32) \
        * (WS * HS)
    dec_wih = inp["dec_wih"].astype(np.float32)
    ydrw = np.zeros((1, 2, 16 * 128), np.float32)
    ydrw[0, 0, :] = dec_wih[:, H] * WS
    ydrw[0, 1, :] = (inp["dec_bih"] + inp["dec_bhh"]).astype(np.float32) \
        * (WS * HS)
    consts = {
        "w12": w12b.astype(nfp8),
        "w3p": w3b.astype(nfp8),
        "wihp": wihp.astype(nfp8),
        "whhp": pack_gate_T(inp["enc_whh"].T.astype(np.float32)),
        "dxwp": pack_gate_T(dec_wih[:, :H].T),
        "ydrw": ydrw.astype(nfp8),
        "dhwp": pack_gate_T(inp["dec_whh"].T.astype(np.float32)),
        "wqt": pack_sq(inp["wq"].T.astype(np.float32)),
        "wkt": pack_sq(inp["wk"].T.astype(np.float32)),
        "wvl": np.ascontiguousarray(
            (inp["wv"].astype(f64).T @ inp["ln_w"].astype(f64).reshape(H)
             * WS).reshape(4, 128).T).astype(nfp8),
    }
    lnb = inp["ln_b"].reshape(1, 1).astype(np.float32)
    return consts, lnb


def kernel(**inputs):
    if not TRACE:
        # NTFF tracing needs antenv.axon_hooks, absent in this container;
        # make sure an inherited BASS_TRACE=1 can't crash the run.
        os.environ["BASS_NEVER_TRACE"] = "1"
    inputs = {k: np.asarray(v) for k, v in inputs.items()}
    if "p1" not in _CACHE:
        _CACHE["p1"] = _build_phase1()
    if "p2" not in _CACHE:
        _CACHE["p2"] = _build_phase2()
    p1, p2 = _CACHE["p1"], _CACHE["p2"]

    consts, lnb = _prep_consts(inputs)
    x = inputs["x"].astype(nfp8)
    y = inputs["y"].astype(np.float32)

    in_maps1 = []
    for c in range(NCORES):
        b0 = c * BL
        ydr_np = np.zeros((1, 2, TP * BL), np.float32)
        ydr_np[0, 0, :] = (y[b0:b0 + BL][:, IDX].T * HS).reshape(-1)
        ydr_np[0, 1, :] = 1.0
        xt = x[b0:b0 + BL].transpose(2, 1, 0)          # [D, T, BL]
        xc = np.zeros((BL // BC, D, T + 1, BC), nfp8)
        for i in range(BL // BC):
            xc[i, :, :T, :] = xt[:, :, i * BC:(i + 1) * BC]
        m = {"x": xc, "ydr": ydr_np.astype(nfp8)}
        m.update(consts)
        in_maps1.append(m)

    r1 = run_bass_kernel_spmd(p1, in_maps1, core_ids=list(range(NCORES)),
                              trace=TRACE)
    LAST_EXEC_NS[0] = r1.exec_time_ns

    # gather k into [p, jtile, k, j] (per-partition contiguous for the DMA)
    kb2 = np.zeros((128, B // 128, 4, 128), nfp8)
    for c in range(NCORES):
        ktc = r1.results[c]["kt"].reshape(4, 128, 4, 128)   # [k, p, i4, j]
        kb2[:, c * 4:(c + 1) * 4, :, :] = ktc.transpose(1, 2, 0, 3)
    vl_full = np.concatenate(
        [r1.results[c]["vl"].astype(np.float32).T.reshape(BL)
         for c in range(NCORES)])
    sv_np = np.zeros((128, B // 128, 33), np.float32)
    sv_np[:, :, 0] = 1.0
    sv_np[:, :, 32] = vl_full.reshape(B // 128, 128).T
    in_maps2 = [
        {"qt": np.ascontiguousarray(
            r1.results[c]["qt"].reshape(4, 128, BL).transpose(1, 0, 2)),
         "kb": kb2, "sv": sv_np.astype(nbf16), "lnb": lnb}
        for c in range(NCORES)
    ]
    r2 = run_bass_kernel_spmd(p2, in_maps2, core_ids=list(range(NCORES)),
                              trace=TRACE)
    LAST_EXEC_NS[1] = r2.exec_time_ns

    out = np.concatenate([r2.results[c]["out"][0] for c in range(NCORES)])
    return out.astype(np.float32)


# revision 22
# speedup vs baseline: 1.5063x; 1.0586x over previous
"""DA-RNN + batch self-attention Trainium2 kernel (8 NeuronCores, SPMD).

Strategy: data-parallel over batch (B=4096 -> 512/core) for CNN + encoder LSTM +
decoder LSTM + q/k/v projections (phase 1).  Host gathers k/v across cores, then
phase 2 computes the BxB softmax attention with score-matrix rows sharded
across cores (each core holds full softmax rows for its 512 queries).

Engine-balance design (cost-model driven):
 - every fp8 matmul runs in DoubleRow mode at psum partition 0; single-k-tile
   passes pair their real rows with a zero/bias slot, and conv matmuls pair
   two (branch, position) outputs per pass through the slot dimension
 - LSTM biases enter through matmul pad slots (constant-1 moving rows), so
   gate activations need no per-m-tile bias and merge into 4-bank-wide ops
 - encoder/decoder steps interleave (enc t || dec t-1) so each LSTM's
   elementwise tail hides under the other's matmuls/activations
 - cell state is bf16 in SBUF (2x DVE rate); conv12 maxpool goes through an
   Activation-engine psum->sbuf copy + strided tensor-tensor max on DVE
 - h3 is branch-packed in partitions with per-branch position shifts so conv3
   is a single block-diagonal 128-partition DR matmul per output position,
   its bias folded into the pad slot and maxpool2 writing featT directly

Self-contained: hardcodes all shapes; takes the full unsharded inputs.
"""

import os
import numpy as np
import ml_dtypes
from contextlib import ExitStack
from itertools import groupby

import concourse.mybir as mybir
import concourse.tile as tile
from concourse import bacc
from concourse.bass_utils import run_bass_kernel_spmd

F32 = mybir.dt.float32
BF16 = mybir.dt.bfloat16
FP8E4 = mybir.dt.float8e4
DR = mybir.MatmulPerfMode.DoubleRow
AF = mybir.ActivationFunctionType
MUL = mybir.AluOpType.mult
ADD = mybir.AluOpType.add
MAX = mybir.AluOpType.max
nbf16 = ml_dtypes.bfloat16
nfp8 = ml_dtypes.float8_e4m3

B, T, D, H, S = 4096, 45, 128, 512, 4
NCORES = 8
BL = B // NCORES          # 512 batch rows per core
BC = 128                  # CNN batch chunk
TP = 9                    # downsampled sequence length
IDX = list(range(T - 1, 0, -(T // TP)))[::-1]   # [4,9,...,44]
NL4 = [18, 8, 4, 2]       # conv3 output positions consumed per branch
NLO = [40, 20, 12, 8]     # conv12 positions needed per branch
T0 = [0, 5, 7, 8]         # featT start index per branch
H3PAD = 20                # h3 pad position (constant 1.0, bias carrier)
FPAD = TP                 # featT pad position (constant 1.0, bias carrier)

WS = 16.0                 # weight prescale
HS = 8.0                  # hidden/feat/y prescale
K3 = 8.0                  # extra conv3/featT scale (better fp8 resolution)
SC = 1.0 / (WS * HS)      # psum -> true preactivation scale
QKS = 4.0                 # extra prescale on stored q/k

# exec times of the two launches from the most recent kernel() call (ns or None)
LAST_EXEC_NS = [None, None]
TRACE = False
_CACHE = {}


def _conv12_plan():
    """Pair-matmul emission plan for conv12.

    psum tile layout: A-tiles [64, 8, BC], global position q = 8g+sub with
    branch 0 at rows 0-31 (conv pos q) and branch 1 at rows 32-63 (conv pos
    q-20, valid q>=20).  B-tile [64, 12, BC]: branch 2 rows 0-31 (pos v),
    branch 3 rows 32-63 (pos v-4, valid v>=4).  The position shifts make
    pooled outputs land at matching h3 positions per branch.

    Returns (vkeys, tiles): vkeys name the stationary-weight variants
    (rebuilt identically on the host); tiles = list of
    (kind, g, nsub, passes), passes = (sub, variant_idx, x_lo, x_step).
    """
    vmap, vkeys = {}, []

    def vi(key):
        if key not in vmap:
            vmap[key] = len(vkeys)
            vkeys.append(key)
        return vmap[key]

    def passes_for(sub, sa, pa, sb=None, pb=None):
        out = []
        if sb is None:
            st = sa + 1
            out.append((sub, vi(("s", sa, 0)), pa * st, st))
            out.append((sub, vi(("s", sa, 2)), (pa + 2) * st, 1))
        else:
            for k in range(3):
                p0 = (pa + k) * (sa + 1)
                p1 = (pb + k) * (sb + 1)
                if p0 < p1:
                    out.append((sub, vi(("p", sa, sb, k, 0)), p0, p1 - p0))
                elif p0 > p1:
                    out.append((sub, vi(("p", sa, sb, k, 1)), p1, p0 - p1))
                else:
                    out.append((sub, vi(("p", sa, sb, k, 2)), p0, 1))
        return out

    tiles = []
    for g in range(5):
        pl = []
        for sub in range(8):
            q = 8 * g + sub
            if q < 20:
                pl += passes_for(sub, 0, q)
            else:
                pl += passes_for(sub, 0, q, 1, q - 20)
        tiles.append(("A", g, 8, pl))
    pl = []
    for sub in range(8):
        if sub < 4:
            pl += passes_for(sub, 2, sub)
        else:
            pl += passes_for(sub, 2, sub, 3, sub - 4)
    tiles.append(("B", 0, 8, pl))
    pl = []
    for sub in range(8, 12):
        pl += passes_for(sub - 8, 2, sub, 3, sub - 4)
    tiles.append(("B2", 0, 4, pl))
    return vkeys, tiles


_VKEYS, _C12TILES = _conv12_plan()
NV12 = len(_VKEYS)


def _build_phase1():
    nc = bacc.Bacc("TRN2", target_bir_lowering=False, debug=False,
                   num_devices=NCORES)
    x = nc.dram_tensor("x", [BL // BC, D, T + 1, BC], FP8E4,
                       kind="ExternalInput")
    ydr = nc.dram_tensor("ydr", [1, 2, TP * BL], FP8E4, kind="ExternalInput")
    w12 = nc.dram_tensor("w12", [128, 2, NV12, 64], FP8E4,
                         kind="ExternalInput")
    w3p = nc.dram_tensor("w3p", [128, 2, 5, 128], FP8E4, kind="ExternalInput")
    wihp = nc.dram_tensor("wihp", [128, 2, 16 * 128], FP8E4,
                          kind="ExternalInput")
    whhp = nc.dram_tensor("whhp", [128, 4, 16 * 128], FP8E4,
                          kind="ExternalInput")
    dxwp = nc.dram_tensor("dxwp", [128, 4, 16 * 128], FP8E4,
                          kind="ExternalInput")
    ydrw = nc.dram_tensor("ydrw", [1, 2, 16 * 128], FP8E4,
                          kind="ExternalInput")
    dhwp = nc.dram_tensor("dhwp", [128, 4, 16 * 128], FP8E4,
                          kind="ExternalInput")
    wqt = nc.dram_tensor("wqt", [128, 4, H], FP8E4, kind="ExternalInput")
    wkt = nc.dram_tensor("wkt", [128, 4, H], FP8E4, kind="ExternalInput")
    wvl = nc.dram_tensor("wvl", [128, 4], FP8E4, kind="ExternalInput")
    qt_d = nc.dram_tensor("qt", [4 * 128, BL], FP8E4, kind="ExternalOutput")
    kt_d = nc.dram_tensor("kt", [4 * 128, BL], FP8E4, kind="ExternalOutput")
    vl_d = nc.dram_tensor("vl", [128, 4], BF16, kind="ExternalOutput")

    with tile.TileContext(nc) as tc, ExitStack() as ctx:
        wpool = ctx.enter_context(tc.tile_pool(name="wpool", bufs=1))
        state = ctx.enter_context(tc.tile_pool(name="state", bufs=1))

        # CNN weights first (conv starts as soon as x chunk 0 lands)
        w12_sb = wpool.tile([128, 2, NV12, 64], FP8E4, tag="w12",
                            name="w12_sb")
        nc.sync.dma_start(out=w12_sb, in_=w12[:, :, :, :])
        w3_sb = wpool.tile([128, 2, 5, 128], FP8E4, tag="w3", name="w3_sb")
        nc.sync.dma_start(out=w3_sb, in_=w3p[:, :, :, :])

        featT = state.tile([128, TP + 1, BL], FP8E4, tag="featT", name="featT")
        nc.gpsimd.memset(featT, 0.0)
        nc.gpsimd.memset(featT[:, FPAD, :], 1.0)
        hencT = state.tile([128, TP, 4, BL], FP8E4, tag="hencT", name="hencT")

        cnnx = ctx.enter_context(tc.tile_pool(name="cnnx", bufs=1))
        xts = []
        for ci in range(BL // BC):
            xT = cnnx.tile([128, T + 1, BC], FP8E4, tag=f"xT{ci}",
                           name=f"xT{ci}")
            nc.sync.dma_start(out=xT[:, 0:12, :], in_=x[ci, :, 0:12, :])
            nc.sync.dma_start(out=xT[:, 12:, :], in_=x[ci, :, 12:, :])
            xts.append(xT)

        # LSTM weights (DMA overlaps the CNN)
        wihp_sb = wpool.tile([128, 2, 16 * 128], FP8E4, tag="wihp",
                             name="wihp_sb")
        nc.sync.dma_start(out=wihp_sb, in_=wihp[:, :, :])
        whhp_sb = wpool.tile([128, 4, 16 * 128], FP8E4, tag="whhp",
                             name="whhp_sb")
        nc.sync.dma_start(out=whhp_sb, in_=whhp[:, :, :])
        ydr_sb = wpool.tile([1, 2, TP * BL], FP8E4, tag="ydr", name="ydr_sb")
        nc.sync.dma_start(out=ydr_sb, in_=ydr[:, :, :])
        dxwp_sb = wpool.tile([128, 4, 16 * 128], FP8E4, tag="dxwp",
                             name="dxwp_sb")
        nc.sync.dma_start(out=dxwp_sb, in_=dxwp[:, :, :])
        ydrw_sb = wpool.tile([1, 2, 16 * 128], FP8E4, tag="ydrw",
                             name="ydrw_sb")
        nc.sync.dma_start(out=ydrw_sb, in_=ydrw[:, :, :])
        dhwp_sb = wpool.tile([128, 4, 16 * 128], FP8E4, tag="dhwp",
                             name="dhwp_sb")
        nc.sync.dma_start(out=dhwp_sb, in_=dhwp[:, :, :])
        wq_sb = wpool.tile([128, 4, H], FP8E4, tag="wq", name="wq_sb")
        nc.sync.dma_start(out=wq_sb, in_=wqt[:, :, :])
        wk_sb = wpool.tile([128, 4, H], FP8E4, tag="wk", name="wk_sb")
        nc.sync.dma_start(out=wk_sb, in_=wkt[:, :, :])
        wvl_sb = wpool.tile([128, 4], FP8E4, tag="wvl", name="wvl_sb")
        nc.sync.dma_start(out=wvl_sb, in_=wvl[:, :])

        # ---------------- CNN downsampling ----------------
        h3s = []
        for ci in range(BL // BC):
            h3 = state.tile([128, H3PAD + 1, BC], FP8E4, tag=f"h3{ci}",
                            name=f"h3_{ci}")
            nc.gpsimd.memset(h3, 0.0)
            nc.gpsimd.memset(h3[:, H3PAD, :], 1.0)
            h3s.append(h3)

        def emit_conv3_mms(ps, o0, no, h3, sub0=0):
            for sub in range(no):
                o = o0 + sub
                rv = 1 + (o >= 10) + (o >= 14) + (o >= 16)
                nc.tensor.matmul(ps[:, sub0 + sub, :], w3_sb[:, :, 0, :],
                                 h3[:, o:o + 2, :], start=True,
                                 stop=False, perf_mode=DR)
                st = H3PAD - o - 2
                nc.tensor.matmul(ps[:, sub0 + sub, :], w3_sb[:, :, rv, :],
                                 h3[:, o + 2:H3PAD + 1:st, :],
                                 start=False, stop=True, perf_mode=DR)

        with (
            tc.tile_pool(name="cpsA", bufs=2, space="PSUM") as cpsA,
            tc.tile_pool(name="cpsB", bufs=1, space="PSUM") as cpsB,
            tc.tile_pool(name="cps3", bufs=1, space="PSUM") as cps3,
            tc.tile_pool(name="hcopy", bufs=3) as hcopy,
        ):
            def ttmax(out, in0, in1):
                nc.vector.tensor_tensor(out, in0, in1, MAX)

            for ci in range(BL // BC):
                xT = xts[ci]
                h3 = h3s[ci]
                cc = slice(ci * BC, (ci + 1) * BC)
                for (kind, g, nsub, passes) in _C12TILES:
                    pool_, tg = (cpsA, "cA") if kind == "A" else (cpsB, "cB")
                    ps = pool_.tile([64, 8, BC], F32, tag=tg,
                                    name=f"c12_{ci}_{kind}{g}")
                    for sub, grp in groupby(passes, key=lambda e: e[0]):
                        grp = list(grp)
                        for idx, (_, v, plo, step) in enumerate(grp):
                            nout = 64 if _VKEYS[v][0] == "p" else 32
                            nc.tensor.matmul(
                                ps[0:nout, sub, :], w12_sb[:, :, v, 0:nout],
                                xT[:, plo:plo + step + 1:step, :],
                                start=(idx == 0), stop=(idx == len(grp) - 1),
                                perf_mode=DR)
                    n2 = nsub
                    if ci == 3:
                        # direct psum reduce on DVE (skips the Act copy);
                        # used where the Act queue is the lead-in bottleneck
                        def red(dst, rows, s0, s1):
                            pv = ps[rows, s0:s1, :].rearrange(
                                "c (l two) b -> c l b two", two=2)
                            nc.vector.tensor_reduce(dst, pv,
                                                    mybir.AxisListType.X, MAX)
                        if kind == "A":
                            if g <= 1:
                                red(h3[0:32, 4 * g:4 * g + 4, :],
                                    slice(0, 32), 0, n2)
                            elif g == 2:
                                red(h3[0:32, 8:12, :], slice(0, 32), 0, n2)
                                red(h3[32:64, 10:12, :], slice(32, 64), 4, n2)
                            else:
                                red(h3[0:64, 4 * g:4 * g + 4, :],
                                    slice(0, 64), 0, n2)
                        elif kind == "B":
                            red(h3[64:96, 14:18, :], slice(0, 32), 0, n2)
                            red(h3[96:128, 16:18, :], slice(32, 64), 4, n2)
                        else:
                            red(h3[64:96, 18:20, :], slice(0, 32), 0, n2)
                            red(h3[96:128, 18:20, :], slice(32, 64), 0, n2)
                        continue
                    hc = hcopy.tile([64, nsub, BC], BF16,
                                    tag=f"hc{kind}", name=f"hc_{ci}_{kind}{g}")
                    nc.scalar.activation(hc, ps[:, 0:nsub, :], AF.Identity)
                    if kind == "A":
                        if g <= 1:
                            ttmax(h3[0:32, 4 * g:4 * g + 4, :],
                                  hc[0:32, 0:n2:2, :], hc[0:32, 1:n2:2, :])
                        elif g == 2:
                            ttmax(h3[0:32, 8:12, :],
                                  hc[0:32, 0:n2:2, :], hc[0:32, 1:n2:2, :])
                            ttmax(h3[32:64, 10:12, :],
                                  hc[32:64, 4:n2:2, :], hc[32:64, 5:n2:2, :])
                        else:
                            ttmax(h3[0:64, 4 * g:4 * g + 4, :],
                                  hc[0:64, 0:n2:2, :], hc[0:64, 1:n2:2, :])
                    elif kind == "B":
                        ttmax(h3[64:96, 14:18, :],
                              hc[0:32, 0:n2:2, :], hc[0:32, 1:n2:2, :])
                        ttmax(h3[96:128, 16:18, :],
                              hc[32:64, 4:n2:2, :], hc[32:64, 5:n2:2, :])
                    else:
                        ttmax(h3[64:96, 18:20, :],
                              hc[0:32, 0:n2:2, :], hc[0:32, 1:n2:2, :])
                        ttmax(h3[96:128, 18:20, :],
                              hc[32:64, 0:n2:2, :], hc[32:64, 1:n2:2, :])
                # conv3 + maxpool2 for featT t 0-3 (blocks 1-2, which
                # fill t 4-8, are deferred into the early LSTM rounds)
                ps = cps3.tile([128, 8, BC], F32, tag="c3",
                               name=f"c3_{ci}_0")
                emit_conv3_mms(ps, 0, 8, h3)
                pv = ps.rearrange("c (l two) b -> c l b two", two=2)
                nc.vector.tensor_reduce(featT[:, 0:4, cc], pv,
                                        mybir.AxisListType.X, MAX)

        # ---------------- interleaved encoder/decoder ----------------
        gpsum = ctx.enter_context(tc.tile_pool(name="gpsum", bufs=2,
                                               space="PSUM"))
        gact = ctx.enter_context(tc.tile_pool(name="gact", bufs=1))
        cpool = ctx.enter_context(tc.tile_pool(name="cpool", bufs=2))
        ttmp = ctx.enter_context(tc.tile_pool(name="ttmp", bufs=3))
        tchp = ctx.enter_context(tc.tile_pool(name="tchp", bufs=2))
        hdp = ctx.enter_context(tc.tile_pool(name="hdp", bufs=2))

        def emit_mms(kind, t, htp, g, ps, rhs_h):
            for j in range(2):
                ht = 2 * htp + j
                cs = slice((4 * g + ht) * 128, (4 * g + ht + 1) * 128)
                if kind == "e":
                    nc.tensor.matmul(
                        ps[:, ht, :], wihp_sb[:, :, cs],
                        featT[:, t:FPAD + 1:FPAD - t, :],
                        start=True, stop=(rhs_h is None), perf_mode=DR)
                else:
                    for k in (0, 2):
                        nc.tensor.matmul(
                            ps[:, ht, :], dxwp_sb[:, k:k + 2, cs],
                            hencT[:, t, k:k + 2, :], start=(k == 0),
                            stop=False, perf_mode=DR)
                    nc.tensor.matmul(
                        ps[:, ht, :], ydrw_sb[:, :, cs],
                        ydr_sb[:, :, t * BL:(t + 1) * BL],
                        start=False, stop=(rhs_h is None), perf_mode=DR)
                if rhs_h is not None:
                    hw_sb = whhp_sb if kind == "e" else dhwp_sb
                    for k in (0, 2):
                        nc.tensor.matmul(
                            ps[:, ht, :], hw_sb[:, k:k + 2, cs],
                            rhs_h[:, k:k + 2, :], start=False,
                            stop=(k == 2), perf_mode=DR)

        def emit_tail(kind, t, sl, c_prev, c_new, acts, h_out):
            if t == 0:
                nc.vector.tensor_tensor(c_new[:, sl, :], acts[0][:, sl, :],
                                        acts[2][:, sl, :], MUL)
            else:
                n = sl.stop - sl.start
                t1 = ttmp.tile([128, n, BL], BF16, tag=f"tt{n}",
                               name=f"t1_{kind}_{t}_{sl.start}")
                nc.vector.tensor_tensor(t1, acts[1][:, sl, :],
                                        c_prev[:, sl, :], MUL)
                t2 = ttmp.tile([128, n, BL], BF16, tag=f"tt{n}",
                               name=f"t2_{kind}_{t}_{sl.start}")
                nc.vector.tensor_tensor(t2, acts[0][:, sl, :],
                                        acts[2][:, sl, :], MUL)
                nc.vector.tensor_tensor(c_new[:, sl, :], t1, t2, ADD)
            n = sl.stop - sl.start
            tch = tchp.tile([128, n, BL], BF16, tag=f"tch{n}",
                            name=f"tch_{kind}_{t}_{sl.start}")
            nc.scalar.activation(tch, c_new[:, sl, :], AF.Tanh)
            nc.vector.scalar_tensor_tensor(h_out[:, sl, :], acts[3][:, sl, :],
                                           HS, tch, MUL, MUL)

        def emit_warm(dep, n):
            wps = gpsum.tile([128, 4, BL], F32, tag="gps", name=f"warm_{warm_i[0]}")
            warm_i[0] += 1
            for i in range(n):
                nc.tensor.matmul(wps[:, i % 4, :], dep[:, 0, 0:128],
                                 dep[:, 0, :], start=True, stop=True)

        warm_i = [0]

        def emit_step(kind, t, rhs_h, c_prev, c_new, h_out, split=False):
            gts = (0, 2, 3) if t == 0 else (0, 1, 2, 3)
            acts = {g: gact.tile([128, 4, BL], BF16, tag=f"{kind}a{g}",
                                 name=f"a_{kind}_{t}_{g}")
                    for g in gts}
            if not split:
                for g in gts:
                    ps = gpsum.tile([128, 4, BL], F32, tag="gps",
                                    name=f"gps_{kind}_{t}_{g}")
                    for htp in (0, 1):
                        emit_mms(kind, t, htp, g, ps, rhs_h)
                    nc.scalar.activation(acts[g], ps,
                                         AF.Tanh if g == 2 else AF.Sigmoid,
                                         scale=SC)
                for htp in (0, 1):
                    emit_tail(kind, t, slice(2 * htp, 2 * htp + 2),
                              c_prev, c_new, acts, h_out)
            else:
                # finer-grained finale: per-gtype acts split in ht halves so
                # the serial tail chain of the last step is shorter
                for g in gts:
                    ps = gpsum.tile([128, 4, BL], F32, tag="gps",
                                    name=f"gps_{kind}_{t}_{g}")
                    for htp in (0, 1):
                        emit_mms(kind, t, htp, g, ps, rhs_h)
                        nc.scalar.activation(
                            acts[g][:, 2 * htp:2 * htp + 2, :],
                            ps[:, 2 * htp:2 * htp + 2, :],
                            AF.Tanh if g == 2 else AF.Sigmoid, scale=SC)
                for htp in (0, 1):
                    emit_tail(kind, t, slice(2 * htp, 2 * htp + 2),
                              c_prev, c_new, acts, h_out)

        def emit_conv3_deferred(ci):
            h3 = h3s[ci]
            cc = slice(ci * BC, (ci + 1) * BC)
            ps = gpsum.tile([128, 16, BC], F32, tag="gps",
                            name=f"c3d_{ci}")
            emit_conv3_mms(ps, 8, 8, h3, sub0=0)
            emit_conv3_mms(ps, 16, 2, h3, sub0=8)
            pv = ps[:, 0:8, :].rearrange("c (l two) b -> c l b two", two=2)
            nc.vector.tensor_reduce(featT[:, 4:8, cc], pv,
                                    mybir.AxisListType.X, MAX)
            pv2 = ps[:, 8:10, :].rearrange("c (l two) b -> c l b two", two=2)
            nc.vector.tensor_reduce(featT[:, 8:9, cc], pv2,
                                    mybir.AxisListType.X, MAX)

        ce_prev = cd_prev = None
        hd_prev = None
        for t in range(TP + 1):
            if 0 < t <= BL // BC:
                emit_conv3_deferred(t - 1)
            if t < TP:
                ce_new = cpool.tile([128, 4, BL], BF16, tag="ce",
                                    name=f"ce_{t}")
                emit_step("e", t, None if t == 0 else hencT[:, t - 1, :, :],
                          ce_prev, ce_new, hencT[:, t, :, :])
                if t == 0:
                    emit_warm(ce_new, 6)
                ce_prev = ce_new
            if t >= 1:
                td = t - 1
                cd_new = cpool.tile([128, 4, BL], BF16, tag="cd",
                                    name=f"cd_{td}")
                hd_new = hdp.tile([128, 4, BL], FP8E4, tag="hd",
                                  name=f"hd_{td}")
                emit_step("d", td, hd_prev, cd_prev, cd_new, hd_new,
                          split=(td == TP - 1))
                if td == TP - 2:
                    emit_warm(cd_new, 6)
                cd_prev, hd_prev = cd_new, hd_new

        # ---------------- q/k/v projections ----------------
        qout = state.tile([128, 4, BL], FP8E4, tag="qout", name="qout")
        kout = state.tile([128, 4, BL], FP8E4, tag="kout", name="kout")
        vlout = state.tile([128, 4], BF16, tag="vlout", name="vlout")
        for w_sb, osb, eng in ((wq_sb, qout, "act"), (wk_sb, kout, "dve")):
            ps = gpsum.tile([128, 4, BL], F32, tag="gps", name=f"qk_{eng}")
            for mh in range(4):
                for k in (0, 2):
                    nc.tensor.matmul(
                        ps[:, mh, :],
                        w_sb[:, k:k + 2, mh * 128:(mh + 1) * 128],
                        hd_prev[:, k:k + 2, :], start=(k == 0),
                        stop=(k == 2), perf_mode=DR)
            if eng == "act":
                nc.scalar.activation(osb, ps, AF.Identity, scale=SC * QKS)
            else:
                nc.vector.tensor_scalar_mul(osb, ps, SC * QKS)
        vlps = gpsum.tile([128, 4, BL], F32, tag="gps", name="vlps")
        for mi in range(4):
            for k in range(4):
                nc.tensor.matmul(vlps[:, 0, mi:mi + 1],
                                 hd_prev[:, k, mi * 128:(mi + 1) * 128],
                                 wvl_sb[:, k:k + 1], start=(k == 0),
                                 stop=(k == 3))
        nc.vector.tensor_scalar_mul(vlout[:, :], vlps[:, 0, 0:4], SC)
        nc.sync.dma_start(out=qt_d.rearrange("(k p) i -> p k i", p=128),
                          in_=qout)
        nc.sync.dma_start(out=kt_d.rearrange("(k p) i -> p k i", p=128),
                          in_=kout)
        nc.sync.dma_start(out=vl_d[:, :], in_=vlout)

    nc.compile()
    return nc


def _build_phase2():
    nc = bacc.Bacc("TRN2", target_bir_lowering=False, debug=False,
                   num_devices=NCORES)
    qt = nc.dram_tensor("qt", [128, 4, BL], FP8E4, kind="ExternalInput")
    kb = nc.dram_tensor("kb", [128, B // 128, 4, 128], FP8E4,
                        kind="ExternalInput")
    sv = nc.dram_tensor("sv", [128, B // 128, 33], BF16, kind="ExternalInput")
    lnb = nc.dram_tensor("lnb", [1, 1], F32, kind="ExternalInput")
    out_d = nc.dram_tensor("out", [1, BL], F32, kind="ExternalOutput")

    NJ = B // 128  # 32 j-tiles of the score matrix
    with tile.TileContext(nc) as tc, ExitStack() as ctx:
        pool = ctx.enter_context(tc.tile_pool(name="p2", bufs=1))
        expool = ctx.enter_context(tc.tile_pool(name="p2e", bufs=3))
        zps = ctx.enter_context(tc.tile_pool(name="zps", bufs=3, space="PSUM"))
        srp = ctx.enter_context(tc.tile_pool(name="srp", bufs=1, space="PSUM"))

        kb_sb = pool.tile([128, NJ, 4, 128], FP8E4, tag="kb", name="kb_sb")
        nc.sync.dma_start(out=kb_sb[:, 0:2, :, :], in_=kb[:, 0:2, :, :])
        qt_sb = pool.tile([128, 4, BL], FP8E4, tag="qt", name="qt_sb")
        nc.sync.dma_start(out=qt_sb, in_=qt[:, :, :])
        nc.sync.dma_start(out=kb_sb[:, 2:4, :, :], in_=kb[:, 2:4, :, :])
        sv_sb = pool.tile([128, NJ, 33], BF16, tag="sv", name="sv_sb")
        nc.sync.dma_start(out=sv_sb, in_=sv[:, :, :])
        lnb_sb = pool.tile([1, 1], F32, tag="lnb", name="lnb_sb")
        nc.sync.dma_start(out=lnb_sb, in_=lnb[:, :])
        for c in range(1, 8):
            nc.sync.dma_start(out=kb_sb[:, 4 * c:4 * (c + 1), :, :],
                              in_=kb[:, 4 * c:4 * (c + 1), :, :])

        # software-pipelined: z-matmuls for pair pi+1 are emitted before the
        # sums/r matmuls of pair pi, so the in-order PE queue never waits on
        # the exp that feeds them
        NP2 = NJ // 2
        sr_ps = srp.tile([33, BL], F32, tag="sr", name="sr_ps")
        exs = [None] * NP2

        def emit_z(pi):
            zp = zps.tile([128, 2, BL], F32, tag="zp", name=f"zp_{pi}")
            for j in range(2):
                tt = 2 * pi + j
                for k in (0, 2):
                    nc.tensor.matmul(zp[:, j, :], kb_sb[:, tt, k:k + 2, :],
                                     qt_sb[:, k:k + 2, :], start=(k == 0),
                                     stop=(k == 2), perf_mode=DR)
            ex = expool.tile([128, 2, BL], BF16, tag="ex", name=f"ex_{pi}")
            nc.scalar.activation(ex, zp, AF.Exp,
                                 scale=float(1.0 / (QKS * QKS * np.sqrt(H))))
            exs[pi] = ex

        def emit_sr(pi):
            for j in range(2):
                nc.tensor.matmul(sr_ps, sv_sb[:, 2 * pi + j, :],
                                 exs[pi][:, j, :],
                                 start=(pi == 0 and j == 0),
                                 stop=(pi == NP2 - 1 and j == 1))

        emit_z(0)
        emit_z(1)
        for pi in range(NP2):
            if pi + 2 < NP2:
                emit_z(pi + 2)
            emit_sr(pi)

        recip = pool.tile([1, BL], F32, tag="recip", name="recip")
        nc.vector.reciprocal(recip, sr_ps[0:1, :])
        prod = pool.tile([1, BL], F32, tag="prod", name="prod")
        nc.vector.tensor_tensor(prod, sr_ps[32:33, :], recip, MUL)
        osb = pool.tile([1, BL], F32, tag="osb", name="osb")
        nc.scalar.activation(osb, prod, AF.Sigmoid, bias=lnb_sb[0:1, 0:1])
        nc.sync.dma_start(out=out_d[:, :], in_=osb)

    nc.compile()
    return nc


def _prep_consts(inp):
    """Host-side weight packing (shared by all cores)."""
    f64 = np.float64
    w1, b1 = inp["rcnn_w1"].astype(f64), inp["rcnn_b1"].astype(f64)
    w2, b2 = inp["rcnn_w2"].astype(f64), inp["rcnn_b2"].astype(f64)
    w3, b3 = inp["rcnn_w3"].astype(f64), inp["rcnn_b3"].astype(f64)
    # fold conv1 (1x1, D->16) into conv2 (3-tap, 16->32):
    w12 = np.einsum("sack,scd->sdka", w2, w1)          # [S, 128, 3, 32]
    b12 = b2 + np.einsum("sack,sc->sa", w2, b1)        # [S, 32]
    # conv2's (folded) bias commutes past the maxpool into conv4's bias
    b3eff = b3 + np.einsum("sack,sc->sa", w3, b12)

    w12b = np.zeros((128, 2, NV12, 64), np.float32)
    for i, key in enumerate(_VKEYS):
        if key[0] == "s":
            _, s, k0 = key
            if k0 == 0:
                w12b[:, 0, i, 0:32] = w12[s, :, 0, :] * WS
                w12b[:, 1, i, 0:32] = w12[s, :, 1, :] * WS
            else:
                w12b[:, 0, i, 0:32] = w12[s, :, 2, :] * WS
        else:
            _, sa, sb, k, order = key
            wa = w12[sa, :, k, :] * WS
            wb = w12[sb, :, k, :] * WS
            if order == 0:
                w12b[:, 0, i, 0:32] = wa
                w12b[:, 1, i, 32:64] = wb
            elif order == 1:
                w12b[:, 0, i, 32:64] = wb
                w12b[:, 1, i, 0:32] = wa
            else:
                w12b[:, 0, i, 0:32] = wa
                w12b[:, 0, i, 32:64] = wb

    # conv3 block-diagonal stationaries: v0 = taps (0,1); v1..v4 = tap2 +
    # bias covering the first rv branches (invalid positions get no bias)
    w3b = np.zeros((128, 2, 5, 128), np.float32)
    for s in range(S):
        r0 = 32 * s
        for k in (0, 1):
            w3b[r0:r0 + 32, k, 0, r0:r0 + 32] = \
                w3[s].transpose(1, 0, 2)[:, :, k] * (HS * K3 / WS)
        for rv in range(1, 5):
            w3b[r0:r0 + 32, 0, rv, r0:r0 + 32] = \
                w3[s].transpose(1, 0, 2)[:, :, 2] * (HS * K3 / WS)
            if s < rv:
                w3b[r0, 1, rv, r0:r0 + 32] = b3eff[s] * (HS * K3)

    def pack_gate_T(wT):   # [in_f, 2048] -> [128, in_f//128, 2048]
        nk = wT.shape[0] // 128
        return np.ascontiguousarray(
            (wT * WS).reshape(nk, 128, -1).transpose(1, 0, 2)).astype(nfp8)

    def pack_sq(wT):       # [512, N] -> [128, 4, N]
        return np.ascontiguousarray(
            (wT * WS).reshape(4, 128, -1).transpose(1, 0, 2)).astype(nfp8)

    wihp = np.zeros((128, 2, 16 * 128), np.float32)
    wihp[:, 0, :] = inp["enc_wih"].T.astype(np.float32) * (WS / K3)
    wihp[0, 1, :] = (inp["enc_bih"] + inp["enc_bhh"]).astype(np.float32) \
        * (WS * HS)
    dec_wih = inp["dec_wih"].astype(np.float32)
    ydrw = np.zeros((1, 2, 16 * 128), np.float32)
    ydrw[0, 0, :] = dec_wih[:, H] * WS
    ydrw[0, 1, :] = (inp["dec_bih"] + inp["dec_bhh"]).astype(np.float32) \
        * (WS * HS)
    consts = {
        "w12": w12b.astype(nfp8),
        "w3p": w3b.astype(nfp8),
        "wihp": wihp.astype(nfp8),
        "whhp": pack_gate_T(inp["enc_whh"].T.astype(np.float32)),
        "dxwp": pack_gate_T(dec_wih[:, :H].T),
        "ydrw": ydrw.astype(nfp8),
        "dhwp": pack_gate_T(inp["dec_whh"].T.astype(np.float32)),
        "wqt": pack_sq(inp["wq"].T.astype(np.float32)),
        "wkt": pack_sq(inp["wk"].T.astype(np.float32)),
        "wvl": np.ascontiguousarray(
            (inp["wv"].astype(f64).T @ inp["ln_w"].astype(f64).reshape(H)
             * WS).reshape(4, 128).T).astype(nfp8),
    }
    lnb = inp["ln_b"].reshape(1, 1).astype(np.float32)
    return consts, lnb


def kernel(**inputs):
    if not TRACE:
        # NTFF tracing needs antenv.axon_hooks, absent in this container;
        # make sure an inherited BASS_TRACE=1 can't crash the run.
        os.environ["BASS_NEVER_TRACE"] = "1"
    inputs = {k: np.asarray(v) for k, v in inputs.items()}
    if "p1" not in _CACHE:
        _CACHE["p1"] = _build_phase1()
    if "p2" not in _CACHE:
        _CACHE["p2"] = _build_phase2()
    p1, p2 = _CACHE["p1"], _CACHE["p2"]

    consts, lnb = _prep_consts(inputs)
    x = inputs["x"].astype(nfp8)
    y = inputs["y"].astype(np.float32)

    in_maps1 = []
    for c in range(NCORES):
        b0 = c * BL
        ydr_np = np.zeros((1, 2, TP * BL), np.float32)
        ydr_np[0, 0, :] = (y[b0:b0 + BL][:, IDX].T * HS).reshape(-1)
        ydr_np[0, 1, :] = 1.0
        xt = x[b0:b0 + BL].transpose(2, 1, 0)          # [D, T, BL]
        xc = np.zeros((BL // BC, D, T + 1, BC), nfp8)
        for i in range(BL // BC):
            xc[i, :, :T, :] = xt[:, :, i * BC:(i + 1) * BC]
        m = {"x": xc, "ydr": ydr_np.astype(nfp8)}
        m.update(consts)
        in_maps1.append(m)

    r1 = run_bass_kernel_spmd(p1, in_maps1, core_ids=list(range(NCORES)),
                              trace=TRACE)
    LAST_EXEC_NS[0] = r1.exec_time_ns

    # gather k into [p, jtile, k, j] (per-partition contiguous for the DMA)
    kb2 = np.zeros((128, B // 128, 4, 128), nfp8)
    for c in range(NCORES):
        ktc = r1.results[c]["kt"].reshape(4, 128, 4, 128)   # [k, p, i4, j]
        kb2[:, c * 4:(c + 1) * 4, :, :] = ktc.transpose(1, 2, 0, 3)
    vl_full = np.concatenate(
        [r1.results[c]["vl"].astype(np.float32).T.reshape(BL)
         for c in range(NCORES)])
    sv_np = np.zeros((128, B // 128, 33), np.float32)
    sv_np[:, :, 0] = 1.0
    sv_np[:, :, 32] = vl_full.reshape(B // 128, 128).T
    in_maps2 = [
        {"qt": np.ascontiguousarray(
            r1.results[c]["qt"].reshape(4, 128, BL).transpose(1, 0, 2)),
         "kb": kb2, "sv": sv_np.astype(nbf16), "lnb": lnb}
        for c in range(NCORES)
    ]
    r2 = run_bass_kernel_spmd(p2, in_maps2, core_ids=list(range(NCORES)),
                              trace=TRACE)
    LAST_EXEC_NS[1] = r2.exec_time_ns

    out = np.concatenate([r2.results[c]["out"][0] for c in range(NCORES)])
    return out.astype(np.float32)


# revision 28
# speedup vs baseline: 1.5100x; 1.0025x over previous
"""DA-RNN + batch self-attention Trainium2 kernel (8 NeuronCores, SPMD).

Strategy: data-parallel over batch (B=4096 -> 512/core) for CNN + encoder LSTM +
decoder LSTM + q/k/v projections (phase 1).  Host gathers k/v across cores, then
phase 2 computes the BxB softmax attention with score-matrix rows sharded
across cores (each core holds full softmax rows for its 512 queries).

Engine-balance design (cost-model driven):
 - every fp8 matmul runs in DoubleRow mode at psum partition 0; single-k-tile
   passes pair their real rows with a zero/bias slot, and conv matmuls pair
   two (branch, position) outputs per pass through the slot dimension
 - LSTM biases enter through matmul pad slots (constant-1 moving rows), so
   gate activations need no per-m-tile bias and merge into 4-bank-wide ops
 - encoder/decoder steps interleave (enc t || dec t-1) so each LSTM's
   elementwise tail hides under the other's matmuls/activations
 - cell state is bf16 in SBUF (2x DVE rate); conv12 maxpool goes through an
   Activation-engine psum->sbuf copy + strided tensor-tensor max on DVE
 - h3 is branch-packed in partitions with per-branch position shifts so conv3
   is a single block-diagonal 128-partition DR matmul per output position,
   its bias folded into the pad slot and maxpool2 writing featT directly

Self-contained: hardcodes all shapes; takes the full unsharded inputs.
"""

import os
import numpy as np
import ml_dtypes
from contextlib import ExitStack
from itertools import groupby

import concourse.mybir as mybir
import concourse.tile as tile
from concourse import bacc
from concourse.bass_utils import run_bass_kernel_spmd

F32 = mybir.dt.float32
BF16 = mybir.dt.bfloat16
FP8E4 = mybir.dt.float8e4
DR = mybir.MatmulPerfMode.DoubleRow
AF = mybir.ActivationFunctionType
MUL = mybir.AluOpType.mult
ADD = mybir.AluOpType.add
MAX = mybir.AluOpType.max
nbf16 = ml_dtypes.bfloat16
nfp8 = ml_dtypes.float8_e4m3

B, T, D, H, S = 4096, 45, 128, 512, 4
NCORES = 8
BL = B // NCORES          # 512 batch rows per core
BC = 128                  # CNN batch chunk
TP = 9                    # downsampled sequence length
IDX = list(range(T - 1, 0, -(T // TP)))[::-1]   # [4,9,...,44]
NL4 = [18, 8, 4, 2]       # conv3 output positions consumed per branch
NLO = [40, 20, 12, 8]     # conv12 positions needed per branch
T0 = [0, 5, 7, 8]         # featT start index per branch (2*T0 = h3 shift)
H3PAD = 20                # h3 pad position (constant 1.0, bias carrier)
FPAD = TP                 # featT pad position (constant 1.0, bias carrier)

WS = 16.0                 # weight prescale
HS = 8.0                  # hidden/feat/y prescale
K3 = 8.0                  # extra conv3/featT scale (better fp8 resolution)
SC = 1.0 / (WS * HS)      # psum -> true preactivation scale
QKS = 4.0                 # extra prescale on stored q/k

# exec times of the two launches from the most recent kernel() call (ns or None)
LAST_EXEC_NS = [None, None]
TRACE = False
_CACHE = {}


def _conv12_plan():
    """Pair-matmul emission plan for conv12.

    psum tile layout: A-tiles [64, 8, BC], global position q = 8g+sub with
    branch 0 at rows 0-31 (conv pos q) and branch 1 at rows 32-63 (conv pos
    q-20, valid q>=20).  B-tile [64, 12, BC]: branch 2 rows 0-31 (pos v),
    branch 3 rows 32-63 (pos v-4, valid v>=4).  The position shifts make
    pooled outputs land at matching h3 positions per branch.

    Returns (vkeys, tiles): vkeys name the stationary-weight variants
    (rebuilt identically on the host); tiles = list of
    (kind, g, nsub, passes), passes = (sub, variant_idx, x_lo, x_step).
    """
    vmap, vkeys = {}, []

    def vi(key):
        if key not in vmap:
            vmap[key] = len(vkeys)
            vkeys.append(key)
        return vmap[key]

    def passes_for(sub, sa, pa, sb=None, pb=None):
        out = []
        if sb is None:
            st = sa + 1
            out.append((sub, vi(("s", sa, 0)), pa * st, st))
            out.append((sub, vi(("s", sa, 2)), (pa + 2) * st, 1))
        else:
            for k in range(3):
                p0 = (pa + k) * (sa + 1)
                p1 = (pb + k) * (sb + 1)
                if p0 < p1:
                    out.append((sub, vi(("p", sa, sb, k, 0)), p0, p1 - p0))
                elif p0 > p1:
                    out.append((sub, vi(("p", sa, sb, k, 1)), p1, p0 - p1))
                else:
                    out.append((sub, vi(("p", sa, sb, k, 2)), p0, 1))
        return out

    tiles = []
    for g in range(5):
        pl = []
        for sub in range(8):
            q = 8 * g + sub
            if q < 20:
                pl += passes_for(sub, 0, q)
            else:
                pl += passes_for(sub, 0, q, 1, q - 20)
        tiles.append(("A", g, 8, pl))
    pl = []
    for sub in range(8):
        if sub < 4:
            pl += passes_for(sub, 2, sub)
        else:
            pl += passes_for(sub, 2, sub, 3, sub - 4)
    tiles.append(("B", 0, 8, pl))
    pl = []
    for sub in range(8, 12):
        pl += passes_for(sub - 8, 2, sub, 3, sub - 4)
    tiles.append(("B2", 0, 4, pl))
    return vkeys, tiles


_VKEYS, _C12TILES = _conv12_plan()
NV12 = len(_VKEYS)


def _build_phase1():
    nc = bacc.Bacc("TRN2", target_bir_lowering=False, debug=False,
                   num_devices=NCORES)
    x = nc.dram_tensor("x", [BL // BC, D, T + 1, BC], FP8E4,
                       kind="ExternalInput")
    ydr = nc.dram_tensor("ydr", [1, 2, TP * BL], FP8E4, kind="ExternalInput")
    w12 = nc.dram_tensor("w12", [128, 2, NV12, 64], FP8E4,
                         kind="ExternalInput")
    w3p = nc.dram_tensor("w3p", [128, 2, 5, 128], FP8E4, kind="ExternalInput")
    wihp = nc.dram_tensor("wihp", [128, 2, 16 * 128], FP8E4,
                          kind="ExternalInput")
    whhp = nc.dram_tensor("whhp", [128, 4, 16 * 128], FP8E4,
                          kind="ExternalInput")
    dxwp = nc.dram_tensor("dxwp", [128, 4, 16 * 128], FP8E4,
                          kind="ExternalInput")
    ydrw = nc.dram_tensor("ydrw", [1, 2, 16 * 128], FP8E4,
                          kind="ExternalInput")
    dhwp = nc.dram_tensor("dhwp", [128, 4, 16 * 128], FP8E4,
                          kind="ExternalInput")
    wqt = nc.dram_tensor("wqt", [128, 4, H], FP8E4, kind="ExternalInput")
    wkt = nc.dram_tensor("wkt", [128, 4, H], FP8E4, kind="ExternalInput")
    wvl = nc.dram_tensor("wvl", [128, 4], FP8E4, kind="ExternalInput")
    qt_d = nc.dram_tensor("qt", [4 * 128, BL], FP8E4, kind="ExternalOutput")
    kt_d = nc.dram_tensor("kt", [4 * 128, BL], FP8E4, kind="ExternalOutput")
    vl_d = nc.dram_tensor("vl", [128, 4], BF16, kind="ExternalOutput")

    with tile.TileContext(nc) as tc, ExitStack() as ctx:
        wpool = ctx.enter_context(tc.tile_pool(name="wpool", bufs=1))
        state = ctx.enter_context(tc.tile_pool(name="state", bufs=1))

        # CNN weights first (conv starts as soon as x chunk 0 lands)
        w12_sb = wpool.tile([128, 2, NV12, 64], FP8E4, tag="w12",
                            name="w12_sb")
        nc.sync.dma_start(out=w12_sb, in_=w12[:, :, :, :])
        w3_sb = wpool.tile([128, 2, 5, 128], FP8E4, tag="w3", name="w3_sb")
        nc.sync.dma_start(out=w3_sb, in_=w3p[:, :, :, :])

        featT = state.tile([128, TP + 1, BL], FP8E4, tag="featT", name="featT")
        nc.gpsimd.memset(featT, 0.0)
        nc.gpsimd.memset(featT[:, FPAD, :], 1.0)
        hencT = state.tile([128, TP, 4, BL], FP8E4, tag="hencT", name="hencT")

        cnnx = ctx.enter_context(tc.tile_pool(name="cnnx", bufs=1))
        xts = []
        for ci in range(BL // BC):
            xT = cnnx.tile([128, T + 1, BC], FP8E4, tag=f"xT{ci}",
                           name=f"xT{ci}")
            nc.sync.dma_start(out=xT[:, 0:12, :], in_=x[ci, :, 0:12, :])
            nc.sync.dma_start(out=xT[:, 12:, :], in_=x[ci, :, 12:, :])
            xts.append(xT)

        # LSTM weights (DMA overlaps the CNN)
        wihp_sb = wpool.tile([128, 2, 16 * 128], FP8E4, tag="wihp",
                             name="wihp_sb")
        nc.sync.dma_start(out=wihp_sb, in_=wihp[:, :, :])
        whhp_sb = wpool.tile([128, 4, 16 * 128], FP8E4, tag="whhp",
                             name="whhp_sb")
        nc.sync.dma_start(out=whhp_sb, in_=whhp[:, :, :])
        ydr_sb = wpool.tile([1, 2, TP * BL], FP8E4, tag="ydr", name="ydr_sb")
        nc.sync.dma_start(out=ydr_sb, in_=ydr[:, :, :])
        dxwp_sb = wpool.tile([128, 4, 16 * 128], FP8E4, tag="dxwp",
                             name="dxwp_sb")
        nc.sync.dma_start(out=dxwp_sb, in_=dxwp[:, :, :])
        ydrw_sb = wpool.tile([1, 2, 16 * 128], FP8E4, tag="ydrw",
                             name="ydrw_sb")
        nc.sync.dma_start(out=ydrw_sb, in_=ydrw[:, :, :])
        dhwp_sb = wpool.tile([128, 4, 16 * 128], FP8E4, tag="dhwp",
                             name="dhwp_sb")
        nc.sync.dma_start(out=dhwp_sb, in_=dhwp[:, :, :])
        wq_sb = wpool.tile([128, 4, H], FP8E4, tag="wq", name="wq_sb")
        nc.sync.dma_start(out=wq_sb, in_=wqt[:, :, :])
        wk_sb = wpool.tile([128, 4, H], FP8E4, tag="wk", name="wk_sb")
        nc.sync.dma_start(out=wk_sb, in_=wkt[:, :, :])
        wvl_sb = wpool.tile([128, 4], FP8E4, tag="wvl", name="wvl_sb")
        nc.sync.dma_start(out=wvl_sb, in_=wvl[:, :])

        # ---------------- CNN downsampling ----------------
        h3s = []
        for ci in range(BL // BC):
            h3 = state.tile([128, H3PAD + 1, BC], FP8E4, tag=f"h3{ci}",
                            name=f"h3_{ci}")
            nc.gpsimd.memset(h3, 0.0)
            nc.gpsimd.memset(h3[:, H3PAD, :], 1.0)
            h3s.append(h3)

        def emit_conv3_mms(ps, o0, no, h3, sub0=0):
            for sub in range(no):
                o = o0 + sub
                rv = 1 + (o >= 10) + (o >= 14) + (o >= 16)
                nc.tensor.matmul(ps[:, sub0 + sub, :], w3_sb[:, :, 0, :],
                                 h3[:, o:o + 2, :], start=True,
                                 stop=False, perf_mode=DR)
                st = H3PAD - o - 2
                nc.tensor.matmul(ps[:, sub0 + sub, :], w3_sb[:, :, rv, :],
                                 h3[:, o + 2:H3PAD + 1:st, :],
                                 start=False, stop=True, perf_mode=DR)

        with (
            tc.tile_pool(name="cpsA", bufs=2, space="PSUM") as cpsA,
            tc.tile_pool(name="cpsB", bufs=1, space="PSUM") as cpsB,
            tc.tile_pool(name="cps3", bufs=1, space="PSUM") as cps3,
            tc.tile_pool(name="hcopy", bufs=3) as hcopy,
        ):
            def ttmax(out, in0, in1):
                nc.vector.tensor_tensor(out, in0, in1, MAX)

            for ci in range(BL // BC):
                xT = xts[ci]
                h3 = h3s[ci]
                cc = slice(ci * BC, (ci + 1) * BC)
                for (kind, g, nsub, passes) in _C12TILES:
                    pool_, tg = (cpsA, "cA") if kind == "A" else (cpsB, "cB")
                    ps = pool_.tile([64, 8, BC], F32, tag=tg,
                                    name=f"c12_{ci}_{kind}{g}")
                    for sub, grp in groupby(passes, key=lambda e: e[0]):
                        grp = list(grp)
                        for idx, (_, v, plo, step) in enumerate(grp):
                            nout = 64 if _VKEYS[v][0] == "p" else 32
                            nc.tensor.matmul(
                                ps[0:nout, sub, :], w12_sb[:, :, v, 0:nout],
                                xT[:, plo:plo + step + 1:step, :],
                                start=(idx == 0), stop=(idx == len(grp) - 1),
                                perf_mode=DR)
                    n2 = nsub
                    hc = hcopy.tile([64, nsub, BC], BF16,
                                    tag=f"hc{kind}", name=f"hc_{ci}_{kind}{g}")
                    nc.scalar.activation(hc, ps[:, 0:nsub, :], AF.Identity)
                    if kind == "A":
                        if g <= 1:
                            ttmax(h3[0:32, 4 * g:4 * g + 4, :],
                                  hc[0:32, 0:n2:2, :], hc[0:32, 1:n2:2, :])
                        elif g == 2:
                            ttmax(h3[0:32, 8:12, :],
                                  hc[0:32, 0:n2:2, :], hc[0:32, 1:n2:2, :])
                            ttmax(h3[32:64, 10:12, :],
                                  hc[32:64, 4:n2:2, :], hc[32:64, 5:n2:2, :])
                        else:
                            ttmax(h3[0:64, 4 * g:4 * g + 4, :],
                                  hc[0:64, 0:n2:2, :], hc[0:64, 1:n2:2, :])
                    elif kind == "B":
                        ttmax(h3[64:96, 14:18, :],
                              hc[0:32, 0:n2:2, :], hc[0:32, 1:n2:2, :])
                        ttmax(h3[96:128, 16:18, :],
                              hc[32:64, 4:n2:2, :], hc[32:64, 5:n2:2, :])
                    else:
                        ttmax(h3[64:96, 18:20, :],
                              hc[0:32, 0:n2:2, :], hc[0:32, 1:n2:2, :])
                        ttmax(h3[96:128, 18:20, :],
                              hc[32:64, 0:n2:2, :], hc[32:64, 1:n2:2, :])
                # conv3 + maxpool2 for featT t 0-3 (blocks 1-2, which
                # fill t 4-8, are deferred into the early LSTM rounds)
                ps = cps3.tile([128, 8, BC], F32, tag="c3",
                               name=f"c3_{ci}_0")
                emit_conv3_mms(ps, 0, 8, h3)
                pv = ps.rearrange("c (l two) b -> c l b two", two=2)
                nc.vector.tensor_reduce(featT[:, 0:4, cc], pv,
                                        mybir.AxisListType.X, MAX)

        # ---------------- interleaved encoder/decoder ----------------
        gpsum = ctx.enter_context(tc.tile_pool(name="gpsum", bufs=2,
                                               space="PSUM"))
        gact = ctx.enter_context(tc.tile_pool(name="gact", bufs=1))
        cpool = ctx.enter_context(tc.tile_pool(name="cpool", bufs=2))
        ttmp = ctx.enter_context(tc.tile_pool(name="ttmp", bufs=3))
        tchp = ctx.enter_context(tc.tile_pool(name="tchp", bufs=2))
        hdp = ctx.enter_context(tc.tile_pool(name="hdp", bufs=2))

        def emit_mms(kind, t, htp, g, ps, rhs_h):
            for j in range(2):
                ht = 2 * htp + j
                cs = slice((4 * g + ht) * 128, (4 * g + ht + 1) * 128)
                if kind == "e":
                    nc.tensor.matmul(
                        ps[:, ht, :], wihp_sb[:, :, cs],
                        featT[:, t:FPAD + 1:FPAD - t, :],
                        start=True, stop=(rhs_h is None), perf_mode=DR)
                else:
                    for k in (0, 2):
                        nc.tensor.matmul(
                            ps[:, ht, :], dxwp_sb[:, k:k + 2, cs],
                            hencT[:, t, k:k + 2, :], start=(k == 0),
                            stop=False, perf_mode=DR)
                    nc.tensor.matmul(
                        ps[:, ht, :], ydrw_sb[:, :, cs],
                        ydr_sb[:, :, t * BL:(t + 1) * BL],
                        start=False, stop=(rhs_h is None), perf_mode=DR)
                if rhs_h is not None:
                    hw_sb = whhp_sb if kind == "e" else dhwp_sb
                    for k in (0, 2):
                        nc.tensor.matmul(
                            ps[:, ht, :], hw_sb[:, k:k + 2, cs],
                            rhs_h[:, k:k + 2, :], start=False,
                            stop=(k == 2), perf_mode=DR)

        def emit_tail(kind, t, sl, c_prev, c_new, acts, h_out):
            if t == 0:
                nc.vector.tensor_tensor(c_new[:, sl, :], acts[0][:, sl, :],
                                        acts[2][:, sl, :], MUL)
            else:
                n = sl.stop - sl.start
                t1 = ttmp.tile([128, n, BL], BF16, tag=f"tt{n}",
                               name=f"t1_{kind}_{t}_{sl.start}")
                nc.vector.tensor_tensor(t1, acts[1][:, sl, :],
                                        c_prev[:, sl, :], MUL)
                t2 = ttmp.tile([128, n, BL], BF16, tag=f"tt{n}",
                               name=f"t2_{kind}_{t}_{sl.start}")
                nc.vector.tensor_tensor(t2, acts[0][:, sl, :],
                                        acts[2][:, sl, :], MUL)
                nc.vector.tensor_tensor(c_new[:, sl, :], t1, t2, ADD)
            n = sl.stop - sl.start
            tch = tchp.tile([128, n, BL], BF16, tag=f"tch{n}",
                            name=f"tch_{kind}_{t}_{sl.start}")
            nc.scalar.activation(tch, c_new[:, sl, :], AF.Tanh)
            nc.vector.scalar_tensor_tensor(h_out[:, sl, :], acts[3][:, sl, :],
                                           HS, tch, MUL, MUL)

        def emit_step(kind, t, rhs_h, c_prev, c_new, h_out, split=False):
            gts = (0, 2, 3) if t == 0 else (0, 1, 2, 3)
            acts = {g: gact.tile([128, 4, BL], BF16, tag=f"{kind}a{g}",
                                 name=f"a_{kind}_{t}_{g}")
                    for g in gts}
            if kind == "e" and t == 0:
                # chunk the t=0 encoder along batch columns so its gate work
                # starts as soon as each CNN chunk's featT lands
                for g in gts:
                    ps = gpsum.tile([128, 4, BL], F32, tag="gps",
                                    name=f"gps_e0_{g}")
                    for ci in range(BL // BC):
                        cc = slice(ci * BC, (ci + 1) * BC)
                        for ht in range(4):
                            cs = slice((4 * g + ht) * 128,
                                       (4 * g + ht + 1) * 128)
                            nc.tensor.matmul(
                                ps[:, ht, cc], wihp_sb[:, :, cs],
                                featT[:, 0:FPAD + 1:FPAD, cc],
                                start=True, stop=True, perf_mode=DR)
                        nc.scalar.activation(acts[g][:, :, cc],
                                             ps[:, :, cc],
                                             AF.Tanh if g == 2 else
                                             AF.Sigmoid, scale=SC)
                for htp in (0, 1):
                    emit_tail(kind, t, slice(2 * htp, 2 * htp + 2),
                              c_prev, c_new, acts, h_out)
                return
            if not split:
                for g in gts:
                    ps = gpsum.tile([128, 4, BL], F32, tag="gps",
                                    name=f"gps_{kind}_{t}_{g}")
                    for htp in (0, 1):
                        emit_mms(kind, t, htp, g, ps, rhs_h)
                    nc.scalar.activation(acts[g], ps,
                                         AF.Tanh if g == 2 else AF.Sigmoid,
                                         scale=SC)
                for htp in (0, 1):
                    emit_tail(kind, t, slice(2 * htp, 2 * htp + 2),
                              c_prev, c_new, acts, h_out)
            else:
                # finer-grained finale: per-gtype acts split in ht halves so
                # the serial tail chain of the last step is shorter
                for g in gts:
                    ps = gpsum.tile([128, 4, BL], F32, tag="gps",
                                    name=f"gps_{kind}_{t}_{g}")
                    for htp in (0, 1):
                        emit_mms(kind, t, htp, g, ps, rhs_h)
                        nc.scalar.activation(
                            acts[g][:, 2 * htp:2 * htp + 2, :],
                            ps[:, 2 * htp:2 * htp + 2, :],
                            AF.Tanh if g == 2 else AF.Sigmoid, scale=SC)
                for htp in (0, 1):
                    emit_tail(kind, t, slice(2 * htp, 2 * htp + 2),
                              c_prev, c_new, acts, h_out)

        def emit_conv3_deferred(ci):
            h3 = h3s[ci]
            cc = slice(ci * BC, (ci + 1) * BC)
            ps = gpsum.tile([128, 16, BC], F32, tag="gps",
                            name=f"c3d_{ci}")
            emit_conv3_mms(ps, 8, 8, h3, sub0=0)
            emit_conv3_mms(ps, 16, 2, h3, sub0=8)
            pv = ps[:, 0:8, :].rearrange("c (l two) b -> c l b two", two=2)
            nc.vector.tensor_reduce(featT[:, 4:8, cc], pv,
                                    mybir.AxisListType.X, MAX)
            pv2 = ps[:, 8:10, :].rearrange("c (l two) b -> c l b two", two=2)
            nc.vector.tensor_reduce(featT[:, 8:9, cc], pv2,
                                    mybir.AxisListType.X, MAX)

        ce_prev = cd_prev = None
        hd_prev = None
        for t in range(TP + 1):
            if 0 < t <= BL // BC:
                emit_conv3_deferred(t - 1)
            if t < TP:
                ce_new = cpool.tile([128, 4, BL], BF16, tag="ce",
                                    name=f"ce_{t}")
                emit_step("e", t, None if t == 0 else hencT[:, t - 1, :, :],
                          ce_prev, ce_new, hencT[:, t, :, :])
                ce_prev = ce_new
            if t >= 1:
                td = t - 1
                cd_new = cpool.tile([128, 4, BL], BF16, tag="cd",
                                    name=f"cd_{td}")
                hd_new = hdp.tile([128, 4, BL], FP8E4, tag="hd",
                                  name=f"hd_{td}")
                emit_step("d", td, hd_prev, cd_prev, cd_new, hd_new,
                          split=(td == TP - 1))
                cd_prev, hd_prev = cd_new, hd_new

        # ---------------- q/k/v projections ----------------
        qout = state.tile([128, 4, BL], FP8E4, tag="qout", name="qout")
        kout = state.tile([128, 4, BL], FP8E4, tag="kout", name="kout")
        vlout = state.tile([128, 4], BF16, tag="vlout", name="vlout")
        for w_sb, osb, eng in ((wq_sb, qout, "act"), (wk_sb, kout, "dve")):
            ps = gpsum.tile([128, 4, BL], F32, tag="gps", name=f"qk_{eng}")
            for mh in range(4):
                for k in (0, 2):
                    nc.tensor.matmul(
                        ps[:, mh, :],
                        w_sb[:, k:k + 2, mh * 128:(mh + 1) * 128],
                        hd_prev[:, k:k + 2, :], start=(k == 0),
                        stop=(k == 2), perf_mode=DR)
            if eng == "act":
                nc.scalar.activation(osb, ps, AF.Identity, scale=SC * QKS)
            else:
                nc.vector.tensor_scalar_mul(osb, ps, SC * QKS)
        vlps = gpsum.tile([128, 4, BL], F32, tag="gps", name="vlps")
        for mi in range(4):
            for k in range(4):
                nc.tensor.matmul(vlps[:, 0, mi:mi + 1],
                                 hd_prev[:, k, mi * 128:(mi + 1) * 128],
                                 wvl_sb[:, k:k + 1], start=(k == 0),
                                 stop=(k == 3))
        nc.vector.tensor_scalar_mul(vlout[:, :], vlps[:, 0, 0:4], SC)
        nc.sync.dma_start(out=qt_d.rearrange("(k p) i -> p k i", p=128),
                          in_=qout)
        nc.sync.dma_start(out=kt_d.rearrange("(k p) i -> p k i", p=128),
                          in_=kout)
        nc.sync.dma_start(out=vl_d[:, :], in_=vlout)

    nc.compile()
    return nc


def _build_phase2():
    nc = bacc.Bacc("TRN2", target_bir_lowering=False, debug=False,
                   num_devices=NCORES)
    qt = nc.dram_tensor("qt", [128, 4, BL], FP8E4, kind="ExternalInput")
    kb = nc.dram_tensor("kb", [128, B // 128, 4, 128], FP8E4,
                        kind="ExternalInput")
    sv = nc.dram_tensor("sv", [128, B // 128, 33], BF16, kind="ExternalInput")
    lnb = nc.dram_tensor("lnb", [1, 1], F32, kind="ExternalInput")
    out_d = nc.dram_tensor("out", [1, BL], F32, kind="ExternalOutput")

    NJ = B // 128  # 32 j-tiles of the score matrix
    with tile.TileContext(nc) as tc, ExitStack() as ctx:
        pool = ctx.enter_context(tc.tile_pool(name="p2", bufs=1))
        expool = ctx.enter_context(tc.tile_pool(name="p2e", bufs=3))
        zps = ctx.enter_context(tc.tile_pool(name="zps", bufs=3, space="PSUM"))
        srp = ctx.enter_context(tc.tile_pool(name="srp", bufs=1, space="PSUM"))

        kb_sb = pool.tile([128, NJ, 4, 128], FP8E4, tag="kb", name="kb_sb")
        nc.sync.dma_start(out=kb_sb[:, 0:2, :, :], in_=kb[:, 0:2, :, :])
        qt_sb = pool.tile([128, 4, BL], FP8E4, tag="qt", name="qt_sb")
        nc.sync.dma_start(out=qt_sb, in_=qt[:, :, :])
        nc.sync.dma_start(out=kb_sb[:, 2:4, :, :], in_=kb[:, 2:4, :, :])
        sv_sb = pool.tile([128, NJ, 33], BF16, tag="sv", name="sv_sb")
        nc.sync.dma_start(out=sv_sb, in_=sv[:, :, :])
        lnb_sb = pool.tile([1, 1], F32, tag="lnb", name="lnb_sb")
        nc.sync.dma_start(out=lnb_sb, in_=lnb[:, :])
        for c in range(1, 8):
            nc.sync.dma_start(out=kb_sb[:, 4 * c:4 * (c + 1), :, :],
                              in_=kb[:, 4 * c:4 * (c + 1), :, :])

        # software-pipelined: z-matmuls for pair pi+1 are emitted before the
        # sums/r matmuls of pair pi, so the in-order PE queue never waits on
        # the exp that feeds them
        NP2 = NJ // 2
        sr_ps = srp.tile([33, BL], F32, tag="sr", name="sr_ps")
        exs = [None] * NP2

        def emit_z(pi):
            zp = zps.tile([128, 2, BL], F32, tag="zp", name=f"zp_{pi}")
            for j in range(2):
                tt = 2 * pi + j
                for k in (0, 2):
                    nc.tensor.matmul(zp[:, j, :], kb_sb[:, tt, k:k + 2, :],
                                     qt_sb[:, k:k + 2, :], start=(k == 0),
                                     stop=(k == 2), perf_mode=DR)
            ex = expool.tile([128, 2, BL], BF16, tag="ex", name=f"ex_{pi}")
            nc.scalar.activation(ex, zp, AF.Exp,
                                 scale=float(1.0 / (QKS * QKS * np.sqrt(H))))
            exs[pi] = ex

        def emit_sr(pi):
            for j in range(2):
                nc.tensor.matmul(sr_ps, sv_sb[:, 2 * pi + j, :],
                                 exs[pi][:, j, :],
                                 start=(pi == 0 and j == 0),
                                 stop=(pi == NP2 - 1 and j == 1))

        emit_z(0)
        emit_z(1)
        for pi in range(NP2):
            if pi + 2 < NP2:
                emit_z(pi + 2)
            emit_sr(pi)

        recip = pool.tile([1, BL], F32, tag="recip", name="recip")
        nc.vector.reciprocal(recip, sr_ps[0:1, :])
        prod = pool.tile([1, BL], F32, tag="prod", name="prod")
        nc.vector.tensor_tensor(prod, sr_ps[32:33, :], recip, MUL)
        osb = pool.tile([1, BL], F32, tag="osb", name="osb")
        nc.scalar.activation(osb, prod, AF.Sigmoid, bias=lnb_sb[0:1, 0:1])
        nc.sync.dma_start(out=out_d[:, :], in_=osb)

    nc.compile()
    return nc


def _prep_consts(inp):
    """Host-side weight packing (shared by all cores)."""
    f64 = np.float64
    w1, b1 = inp["rcnn_w1"].astype(f64), inp["rcnn_b1"].astype(f64)
    w2, b2 = inp["rcnn_w2"].astype(f64), inp["rcnn_b2"].astype(f64)
    w3, b3 = inp["rcnn_w3"].astype(f64), inp["rcnn_b3"].astype(f64)
    # fold conv1 (1x1, D->16) into conv2 (3-tap, 16->32):
    w12 = np.einsum("sack,scd->sdka", w2, w1)          # [S, 128, 3, 32]
    b12 = b2 + np.einsum("sack,sc->sa", w2, b1)        # [S, 32]
    # conv2's (folded) bias commutes past the maxpool into conv4's bias
    b3eff = b3 + np.einsum("sack,sc->sa", w3, b12)

    w12b = np.zeros((128, 2, NV12, 64), np.float32)
    for i, key in enumerate(_VKEYS):
        if key[0] == "s":
            _, s, k0 = key
            if k0 == 0:
                w12b[:, 0, i, 0:32] = w12[s, :, 0, :] * WS
                w12b[:, 1, i, 0:32] = w12[s, :, 1, :] * WS
            else:
                w12b[:, 0, i, 0:32] = w12[s, :, 2, :] * WS
        else:
            _, sa, sb, k, order = key
            wa = w12[sa, :, k, :] * WS
            wb = w12[sb, :, k, :] * WS
            if order == 0:
                w12b[:, 0, i, 0:32] = wa
                w12b[:, 1, i, 32:64] = wb
            elif order == 1:
                w12b[:, 0, i, 32:64] = wb
                w12b[:, 1, i, 0:32] = wa
            else:
                w12b[:, 0, i, 0:32] = wa
                w12b[:, 0, i, 32:64] = wb

    # conv3 block-diagonal stationaries: v0 = taps (0,1); v1..v4 = tap2 +
    # bias covering the first rv branches (invalid positions get no bias)
    w3b = np.zeros((128, 2, 5, 128), np.float32)
    for s in range(S):
        r0 = 32 * s
        for k in (0, 1):
            w3b[r0:r0 + 32, k, 0, r0:r0 + 32] = \
                w3[s].transpose(1, 0, 2)[:, :, k] * (HS * K3 / WS)
        for rv in range(1, 5):
            w3b[r0:r0 + 32, 0, rv, r0:r0 + 32] = \
                w3[s].transpose(1, 0, 2)[:, :, 2] * (HS * K3 / WS)
            if s < rv:
                w3b[r0, 1, rv, r0:r0 + 32] = b3eff[s] * (HS * K3)

    def pack_gate_T(wT):   # [in_f, 2048] -> [128, in_f//128, 2048]
        nk = wT.shape[0] // 128
        return np.ascontiguousarray(
            (wT * WS).reshape(nk, 128, -1).transpose(1, 0, 2)).astype(nfp8)

    def pack_sq(wT):       # [512, N] -> [128, 4, N]
        return np.ascontiguousarray(
            (wT * WS).reshape(4, 128, -1).transpose(1, 0, 2)).astype(nfp8)

    wihp = np.zeros((128, 2, 16 * 128), np.float32)
    wihp[:, 0, :] = inp["enc_wih"].T.astype(np.float32) * (WS / K3)
    wihp[0, 1, :] = (inp["enc_bih"] + inp["enc_bhh"]).astype(np.float32) \
        * (WS * HS)
    dec_wih = inp["dec_wih"].astype(np.float32)
    ydrw = np.zeros((1, 2, 16 * 128), np.float32)
    ydrw[0, 0, :] = dec_wih[:, H] * WS
    ydrw[0, 1, :] = (inp["dec_bih"] + inp["dec_bhh"]).astype(np.float32) \
        * (WS * HS)
    consts = {
        "w12": w12b.astype(nfp8),
        "w3p": w3b.astype(nfp8),
        "wihp": wihp.astype(nfp8),
        "whhp": pack_gate_T(inp["enc_whh"].T.astype(np.float32)),
        "dxwp": pack_gate_T(dec_wih[:, :H].T),
        "ydrw": ydrw.astype(nfp8),
        "dhwp": pack_gate_T(inp["dec_whh"].T.astype(np.float32)),
        "wqt": pack_sq(inp["wq"].T.astype(np.float32)),
        "wkt": pack_sq(inp["wk"].T.astype(np.float32)),
        "wvl": np.ascontiguousarray(
            (inp["wv"].astype(f64).T @ inp["ln_w"].astype(f64).reshape(H)
             * WS).reshape(4, 128).T).astype(nfp8),
    }
    lnb = inp["ln_b"].reshape(1, 1).astype(np.float32)
    return consts, lnb


def kernel(**inputs):
    if not TRACE:
        # NTFF tracing needs antenv.axon_hooks, absent in this container;
        # make sure an inherited BASS_TRACE=1 can't crash the run.
        os.environ["BASS_NEVER_TRACE"] = "1"
    inputs = {k: np.asarray(v) for k, v in inputs.items()}
    if "p1" not in _CACHE:
        _CACHE["p1"] = _build_phase1()
    if "p2" not in _CACHE:
        _CACHE["p2"] = _build_phase2()
    p1, p2 = _CACHE["p1"], _CACHE["p2"]

    consts, lnb = _prep_consts(inputs)
    x = inputs["x"].astype(nfp8)
    y = inputs["y"].astype(np.float32)

    in_maps1 = []
    for c in range(NCORES):
        b0 = c * BL
        ydr_np = np.zeros((1, 2, TP * BL), np.float32)
        ydr_np[0, 0, :] = (y[b0:b0 + BL][:, IDX].T * HS).reshape(-1)
        ydr_np[0, 1, :] = 1.0
        xt = x[b0:b0 + BL].transpose(2, 1, 0)          # [D, T, BL]
        xc = np.zeros((BL // BC, D, T + 1, BC), nfp8)
        for i in range(BL // BC):
            xc[i, :, :T, :] = xt[:, :, i * BC:(i + 1) * BC]
        m = {"x": xc, "ydr": ydr_np.astype(nfp8)}
        m.update(consts)
        in_maps1.append(m)

    r1 = run_bass_kernel_spmd(p1, in_maps1, core_ids=list(range(NCORES)),
                              trace=TRACE)
    LAST_EXEC_NS[0] = r1.exec_time_ns

    # gather k into [p, jtile, k, j] (per-partition contiguous for the DMA)
    kb2 = np.zeros((128, B // 128, 4, 128), nfp8)
    for c in range(NCORES):
        ktc = r1.results[c]["kt"].reshape(4, 128, 4, 128)   # [k, p, i4, j]
        kb2[:, c * 4:(c + 1) * 4, :, :] = ktc.transpose(1, 2, 0, 3)
    vl_full = np.concatenate(
        [r1.results[c]["vl"].astype(np.float32).T.reshape(BL)
         for c in range(NCORES)])
    sv_np = np.zeros((128, B // 128, 33), np.float32)
    sv_np[:, :, 0] = 1.0
    sv_np[:, :, 32] = vl_full.reshape(B // 128, 128).T
    in_maps2 = [
        {"qt": np.ascontiguousarray(
            r1.results[c]["qt"].reshape(4, 128, BL).transpose(1, 0, 2)),
         "kb": kb2, "sv": sv_np.astype(nbf16), "lnb": lnb}
        for c in range(NCORES)
    ]
    r2 = run_bass_kernel_spmd(p2, in_maps2, core_ids=list(range(NCORES)),
                              trace=TRACE)
    LAST_EXEC_NS[1] = r2.exec_time_ns

    out = np.concatenate([r2.results[c]["out"][0] for c in range(NCORES)])
    return out.astype(np.float32)
